# revision 42
# baseline (speedup 1.0000x reference)
"""ANT_Linear fused kernel for 8 TRN2 NeuronCores (raw Bass, manual sems).

out = fakequant(x) @ W.T + bias; per-128-group absmax scaling of x snapped to
the 15-level e2m1 ('flint') grid.  Data-parallel over tokens: 2048/core,
16 tiles of [128 tokens, 4096 features].

v3: PE-saturating schedule with a signed magic-add snap:

  xs = RN16(x*(6/absmax))               (Pool broadcast mult)
  t' = copysign(512*2^floor(log2(max(|xs|,1))), xs)
       via uint16-lane bit ops: exp-mask, clamp, +9<<10, or-sign  (DVE)
  c  = RN16(xs + t')                    (DVE: rounds xs at the grid quantum)
  y  = c - t'                           (DVE, exact: y = snap(xs) signed)
  dq = y * scale                        (ACT per-group Copy-with-scale)
  dqT = dma_transpose(dq); out = dqT.T @ W.T + bias on PE (fp16 matmuls)

Engine busy per [128,4096] tile (cost-model ns):
  DVE : reduce 4297 + sc/rr 171 + t_e/t/s 3x1127 + t' 2163 + c/y 2x2194
        -> ~14.7us/tile  <- pipeline bound
  PE  : 64 fp16 matmuls + 2 bias matmuls = 14.1us
  ACT : dq 32x292 + psum->sbuf out copy = 10.4us;  Pool: xs 8.3us
  DMA : x 5.8 + 2 transpose halves 3.6 + out 1.5 = 10.9us (single queue)

Schedule notes:
 - A self-paced warmup stream of dummy matmuls (pairs, self-incrementing
   sem, 2-pair lookahead) keeps the PE engine continuously busy -- and the
   cost model's p-state ramp warm -- until tile0's inputs are resident
   (~51us: the FIFO DMA queue must carry x0..x3 + all weights + bias +
   tile0's transposes first).  Tile0 then starts coarse (all waits up
   front) and the stream stays gap-free; the quant chain runs ~1.5 tiles
   ahead of the PE throughout.
 - tile0's x-load/stats/xs/front/dq are split in halves to shorten the
   fill; the DMA issue order is hand-choreographed (x2 gated on rr0, the
   last 4 weight chunks gated on tile0's transposes, x4 gated on dq(1))
   because the cost model serializes all DMAs on one engine FIFO.
 - DMA completions are NOT ordered across engines: every wait must
   identify one specific DMA.  Hence per-buffer-slot x sems, per-half
   sems for the split x0/x3, h0/h1 transpose sems (sTPa/sTPb), out-DMA
   parity sems (sODa/sODb), and predecessor waits before reusing a sem.
 - Iterative DVE ops (tensor_reduce, reciprocal) need an explicit
   drain() before a dependent consumer (HW RAW hazard -- verified: NaNs
   without).  Simple ALU chains (ts/tt) are safe drain-free, and sem
   incs ride on the last compute op instead of a trailing drain.

Numerics: bit-exact with the v2 baseline except the PO2 magic constant
fixes v2's snap at |xs| in {1.997..1.999, 3.994..3.998} (v2 rounded these
to 1.5/3.0 instead of 2.0/4.0) -- rel err improves 1.43e-2 -> 1.11e-2.
"""

import numpy as np

N_CORES = 8
TOK = 4 * 4096
TPC = TOK // N_CORES    # 2048
K = 4096
M = 1024
GS = 128
G = K // GS             # 32
TT = 128
NT = TPC // TT          # 16

WARM = 112              # warmup matmul pairs

_CACHE = {}


def _register_snap_ops():
    """Register two fused custom-DVE ops implementing the signed magic-add
    snap in one pass each (f32 bit tricks inside the DVE pipe):
      ANT_SNAP_C: c = RN16(xs + t'(xs))
      ANT_SNAP_Y: y = c - t'(xs)        (sign taken from c; sign(c)==sign(xs))
    with t'(u) = copysign(512 * 2^floor(log2(max(|u|,1))), u).
    """
    if "snap_ops" in _CACHE:
        return _CACHE["snap_ops"]
    import concourse.dve_ops as dops
    from concourse.dve_spec import (AluOp, Bin, C0, C1, C2, Spec, Src0, Src1,
                                    Zero, _has_src1, lower, maxx, select)
    from concourse.dve_uop import DveOpSpec

    def _tmag_np(src, s1, imm2):
        b = (src.astype(np.float32).view(np.int32) & 0x7F800000).view(
            np.float32)
        return np.maximum(b, np.float32(s1)) * np.float32(imm2)

    def ref_snap_c(in0, in1, s0, s1, imm2):
        x = in0.astype(np.float32)
        t = _tmag_np(x, s1, imm2)
        return x + np.where(x < 0, -t, t).astype(np.float32)

    def ref_snap_y(in0, in1, s0, s1, imm2):
        c = in0.astype(np.float32)
        t = _tmag_np(in1, s1, imm2)
        return c - np.where(c < 0, -t, t).astype(np.float32)

    _tm0 = maxx(Bin(AluOp.BITWISE_AND, Src0, C0), C1) * C2
    spec_c = Spec(body=Src0 + select(Src0 < Zero, Zero - _tm0, _tm0),
                  reference=ref_snap_c)
    _tm1 = maxx(Bin(AluOp.BITWISE_AND, Src1, C0), C1) * C2
    spec_y = Spec(body=Src0 - select(Src0 < Zero, Zero - _tm1, _tm1),
                  reference=ref_snap_y)

    ops = []
    for name, spec in (("ANT_SNAP_C", spec_c), ("ANT_SNAP_Y", spec_y)):
        if name in dops._SUB_OPCODE_FOR_NAME:
            ops.append(next(o for o in dops.OPS if o.name == name))
            continue
        row = max(dops._SUB_OPCODE_FOR_NAME.values()) + 1
        assert row < 0x20
        dops._SUB_OPCODE_FOR_NAME[name] = row
        shas = {}
        for ver in ("v3", "v4"):
            uops = lower(spec, ver=ver)
            sp = DveOpSpec(name=name, opcode=row, uops=uops,
                           rd1_en=_has_src1(spec))
            shas[ver] = sp.sha(ver)
        op = dops.DveOp(name, spec, subdim=False, uops_sha=shas)
        dops.OPS.append(op)
        dops.CUSTOM_DVE_SPECS[name] = spec
        ops.append(op)
    _CACHE["snap_ops"] = tuple(ops)
    return _CACHE["snap_ops"]


def _build_bass(nt=NT):
    from contextlib import ExitStack

    import concourse.bass as bass
    import concourse.mybir as mybir

    dt = mybir.dt
    alu = mybir.AluOpType
    AF = mybir.ActivationFunctionType

    SNAP_C, SNAP_Y = _register_snap_ops()

    NTL = nt
    nc = bass.Bass()
    x_d = nc.declare_dram_parameter("x", [NTL * TT, K], dt.float32, isOutput=False)
    wt_d = nc.declare_dram_parameter("wt", [K, M], dt.float16, isOutput=False)
    b_d = nc.declare_dram_parameter("bias", [1, M], dt.float16, isOutput=False)
    out_d = nc.declare_dram_parameter("out", [NTL * TT, M], dt.float32, isOutput=True)

    x_t4 = x_d.rearrange("(n p) (g s) -> n p g s", p=TT, s=GS)   # [16,128,32,128]
    wt_t3 = wt_d.rearrange("(b p) m -> p b m", p=128)            # [128,32,1024]

    ctx = ExitStack()
    with ctx:
        sb = lambda name, shape, d: ctx.enter_context(nc.sbuf_tensor(name, shape, d))
        ps = lambda name, shape, d: ctx.enter_context(nc.psum_tensor(name, shape, d))
        sem = lambda name: ctx.enter_context(nc.semaphore(name))

        wt_sb = sb("wt_sb", [128, G, M], dt.float16)            # 8 MiB resident
        bias_sb = sb("bias_sb", [1, M], dt.float16)
        ones_sb = sb("ones_sb", [1, TT], dt.float16)

        x_sb = [sb(f"x_sb{k}", [TT, G, GS], dt.float32) for k in range(3)]
        xs_sb = [sb(f"xs_sb{k}", [TT, K], dt.float16) for k in range(2)]
        t_sb = sb("t_sb", [TT, K], dt.float16)
        s_sb = sb("s_sb", [TT, K], dt.float16)
        y_sb = [sb(f"y_sb{k}", [TT, K], dt.float16) for k in range(2)]
        dq_sb = [sb(f"dq_sb{k}", [TT, K], dt.float16) for k in range(2)]
        dqt_sb = [sb(f"dqt_sb{k}", [128, G, TT], dt.float16) for k in range(2)]
        o_sb = [sb(f"o_sb{k}", [TT, M], dt.float32) for k in range(2)]
        warm_sb = sb("warm_sb", [TT, 512], dt.float16)
        amax_sb = [sb(f"amax_sb{k}", [TT, G], dt.float32) for k in range(2)]
        sc_sb = [sb(f"sc_sb{k}", [TT, G], dt.float32) for k in range(4)]
        rr_sb = [sb(f"rr_sb{k}", [TT, G], dt.float32) for k in range(2)]

        pout_ps = [ps(f"pout_ps{k}", [TT, M], dt.float32) for k in range(2)]
        warm_ps = ps("warm_ps", [TT, 512], dt.float32)

        sC = sem("sC")     # bias DMA done (+16)
        sV = sem("sV")     # ones/warm memset done
        # x-load sems: one per buffer slot (+ dedicated sems for the four
        # half-loads of x0/x3) so a sem value always identifies ONE DMA --
        # DMA completions are not ordered across engines.
        sXA = sem("sXA")   # slot0 whole loads: x6, x9, x12, x15
        sXB = sem("sXB")   # slot1 loads: x1, x4, x7, x10, x13
        sXC = sem("sXC")   # slot2 loads: x2, x5, x8, x11, x14
        sX0A = sem("sX0A")  # x0 first half
        sX0B = sem("sX0B")  # x0 second half
        sX3A = sem("sX3A")  # x3 first half
        sX3B = sem("sX3B")  # x3 second half
        sWT = sem("sWT")   # weight chunk in (+16/chunk)
        sRR = sem("sRR")   # stats done (amax/sc/rr) (+1/half for tile0, +2/tile)
        sXS = sem("sXS")   # Pool xs done (+1/half tile0, +2/tile)
        sFR = sem("sFR")   # DVE front chain done (+1/half tile0, +2/tile)
        sDU = sem("sDU")   # ACT dq half done (+16/half)
        sTPa = sem("sTPa")  # transpose h0 done (+16/tile)
        sTPb = sem("sTPb")  # transpose h1 done (+16/tile)
        sMM = sem("sMM")   # PE tile done (+1/tile)
        sOC = sem("sOC")   # ACT out copy done (+1/tile)
        sODa = sem("sODa")  # out DMA done, even tiles (+16)
        sODb = sem("sODb")  # out DMA done, odd tiles (+16)
        sWU = sem("sWU")   # warmup pacing (+1/warmup mm)

        def x_sem(i):
            """(sem, value) identifying completion of whole-load x(i)."""
            if i % 3 == 0:
                return sXA, 16 * (i // 3 - 1)
            if i % 3 == 1:
                return sXB, 16 * ((i - 1) // 3 + 1)
            return sXC, 16 * ((i - 2) // 3 + 1)

        def wait_x(eng, j):
            """Wait for x(j) fully loaded."""
            if j == 0:
                eng.wait_ge(sX0A, 16)
                eng.wait_ge(sX0B, 16)
            elif j == 3:
                eng.wait_ge(sX3A, 16)
                eng.wait_ge(sX3B, 16)
            else:
                s, v = x_sem(j)
                eng.wait_ge(s, v)

        xs_i32 = [xs_sb[k].bitcast(dt.int32) for k in range(2)]
        xs_u16 = [xs_sb[k].bitcast(dt.uint16) for k in range(2)]
        t_i32 = t_sb.bitcast(dt.int32)
        t_u16 = t_sb.bitcast(dt.uint16)
        s_i32 = s_sb.bitcast(dt.int32)
        y3 = [y_sb[k].rearrange("p (g s) -> p g s", s=GS) for k in range(2)]
        dq3 = [dq_sb[k].rearrange("p (g s) -> p g s", s=GS) for k in range(2)]

        def wait_tp(eng, j, full):
            """Wait for tile j's transpose h0 (full=False) or both halves."""
            eng.wait_ge(sTPa, 16 * (j + 1))
            if full:
                eng.wait_ge(sTPb, 16 * (j + 1))

        with nc.Block() as block:

            @block.sync
            def _(eng):
                # FIFO DMA queue choreography (see docstring).
                eng.dma_start(
                    out=x_sb[0][:, 0:16, :], in_=x_t4[0][:, 0:16, :]
                ).then_inc(sX0A, 16)
                eng.dma_start(
                    out=x_sb[0][:, 16:32, :], in_=x_t4[0][:, 16:32, :]
                ).then_inc(sX0B, 16)
                eng.dma_start(out=x_sb[1][:, :, :], in_=x_t4[1]).then_inc(sXB, 16)
                eng.dma_start(out=bias_sb[:, :], in_=b_d[:, :]).then_inc(sC, 16)
                for c in range(2):
                    eng.dma_start(
                        out=wt_sb[:, 4 * c:4 * (c + 1), :],
                        in_=wt_t3[:, 4 * c:4 * (c + 1), :],
                    ).then_inc(sWT, 16)
                eng.wait_ge(sRR, 2)                # x2 right when stats need it
                eng.dma_start(out=x_sb[2][:, :, :], in_=x_t4[2]).then_inc(sXC, 16)
                for c in range(2, 6):
                    eng.dma_start(
                        out=wt_sb[:, 4 * c:4 * (c + 1), :],
                        in_=wt_t3[:, 4 * c:4 * (c + 1), :],
                    ).then_inc(sWT, 16)
                eng.wait_ge(sDU, 16)               # tile0 dq half ready
                eng.dma_start(
                    out=dqt_sb[0][:, 0:16, :], in_=dq_sb[0][:, 0:2048],
                    transpose=True,
                ).then_inc(sTPa, 16)
                eng.wait_ge(sDU, 32)
                eng.dma_start(
                    out=dqt_sb[0][:, 16:32, :], in_=dq_sb[0][:, 2048:4096],
                    transpose=True,
                ).then_inc(sTPb, 16)
                eng.dma_start(
                    out=x_sb[0][:, 0:16, :], in_=x_t4[3][:, 0:16, :]
                ).then_inc(sX3A, 16)
                for c in range(6, 8):
                    eng.dma_start(
                        out=wt_sb[:, 4 * c:4 * (c + 1), :],
                        in_=wt_t3[:, 4 * c:4 * (c + 1), :],
                    ).then_inc(sWT, 16)
                eng.dma_start(
                    out=x_sb[0][:, 16:32, :], in_=x_t4[3][:, 16:32, :]
                ).then_inc(sX3B, 16)
                for i in range(4, NTL):
                    if i == 4:
                        # don't head-of-line block tile1's transposes
                        eng.wait_ge(sDU, 64)
                    eng.wait_ge(sXS, 2 * (i - 2))  # x buf free (x3 ring)
                    s, v = x_sem(i)
                    if v > 16:
                        eng.wait_ge(s, v - 16)     # slot predecessor done
                    eng.dma_start(
                        out=x_sb[i % 3][:, :, :], in_=x_t4[i]
                    ).then_inc(s, 16)

            @block.vector
            def _(eng):
                nc.vector.memset(ones_sb[:, :], 1.0)
                nc.vector.memset(warm_sb[:, :], 0.5)
                nc.vector.drain().then_inc(sV, 1)

                def stats(j, half=None):
                    if j >= NTL:
                        return
                    gl, gh = (0, G) if half is None else (
                        (0, G // 2) if half == 0 else (G // 2, G))
                    if j == 0:
                        eng.wait_ge(sX0A if half == 0 else sX0B, 16)
                    elif j == 3:
                        eng.wait_ge(sX3A if half == 0 else sX3B, 16)
                    else:
                        wait_x(eng, j)
                    if j >= 4:
                        # sc buf (x4): dq(j-4) groups all scaled
                        eng.wait_ge(sDU, 32 * (j - 3))
                    nc.vector.tensor_reduce(
                        out=amax_sb[j % 2][:, gl:gh],
                        in_=x_sb[j % 3][:, gl:gh, :],
                        axis=mybir.AxisListType.X, op=alu.max,
                        apply_absolute_value=True,
                    )
                    nc.vector.drain()
                    # scale = max(amax/6, tiny); rr = 1/scale
                    nc.vector.tensor_scalar(
                        out=sc_sb[j % 4][:, gl:gh], in0=amax_sb[j % 2][:, gl:gh],
                        scalar1=1.0 / 6.0, scalar2=1e-30,
                        op0=alu.mult, op1=alu.max,
                    )
                    nc.vector.drain()
                    nc.vector.reciprocal(
                        out=rr_sb[j % 2][:, gl:gh], in_=sc_sb[j % 4][:, gl:gh]
                    ).then_inc(sRR, 2 if half is None else 1)

                def front(j, half=None):
                    if j < 0 or j >= NTL:
                        return
                    lo, hi = (0, K) if half is None else (
                        (0, K // 2) if half == 0 else (K // 2, K))
                    eng.wait_ge(sXS, 2 * j + 1 + (half != 0))
                    if j >= 2:
                        eng.wait_ge(sDU, 32 * (j - 1))  # y buf free
                    l2, h2 = lo // 2, hi // 2
                    # t_e = exp_bits(xs)
                    nc.vector.tensor_scalar(
                        out=t_u16[:, lo:hi], in0=xs_u16[j % 2][:, lo:hi],
                        scalar1=0x7C00, scalar2=None, op0=alu.bitwise_and,
                    )
                    # t = (max(t_e, bits(1.0)) + 0x2400: PO2 of max(|xs|,1)
                    # times 512, as an exponent-field add
                    nc.vector.tensor_scalar(
                        out=t_u16[:, lo:hi], in0=t_u16[:, lo:hi],
                        scalar1=0x3C00, scalar2=0x2400,
                        op0=alu.max, op1=alu.add,
                    )
                    # s = sign pairs
                    nc.vector.tensor_scalar(
                        out=s_i32[:, l2:h2], in0=xs_i32[j % 2][:, l2:h2],
                        scalar1=-2147450880, scalar2=None, op0=alu.bitwise_and,
                    )
                    # t' = t | s  (copysign, in place over t)
                    nc.vector.tensor_tensor(
                        out=t_i32[:, l2:h2], in0=t_i32[:, l2:h2],
                        in1=s_i32[:, l2:h2], op=alu.bitwise_or,
                    )
                    # c = RN16(xs + t'): rounds xs at the grid quantum
                    nc.vector.tensor_tensor(
                        out=y_sb[j % 2][:, lo:hi], in0=xs_sb[j % 2][:, lo:hi],
                        in1=t_sb[:, lo:hi], op=alu.add,
                    )
                    # y = c - t'  (exact; signed snap)
                    nc.vector.tensor_tensor(
                        out=y_sb[j % 2][:, lo:hi], in0=y_sb[j % 2][:, lo:hi],
                        in1=t_sb[:, lo:hi], op=alu.subtract,
                    ).then_inc(sFR, 2 if half is None else 1)

                def chain(i):
                    """Iteration i: sc/recip of tile (i+1) (their producer
                    reduce ran LAST iteration -- no drain needed), front(i)
                    ops interleaved as separators, reduce(i+2) at the end."""
                    nr = i + 1 < NTL
                    if nr:
                        nc.vector.tensor_scalar(
                            out=sc_sb[(i + 1) % 4][:, :],
                            in0=amax_sb[(i + 1) % 2][:, :],
                            scalar1=1.0 / 6.0, scalar2=1e-30,
                            op0=alu.mult, op1=alu.max,
                        )
                    eng.wait_ge(sXS, 2 * i + 2)
                    if i >= 2:
                        eng.wait_ge(sDU, 32 * (i - 1))   # y buf free
                    nc.vector.tensor_scalar(
                        out=t_u16[:, :], in0=xs_u16[i % 2][:, :],
                        scalar1=0x7C00, scalar2=None, op0=alu.bitwise_and,
                    )
                    nc.vector.tensor_scalar(
                        out=t_u16[:, :], in0=t_u16[:, :],
                        scalar1=0x3C00, scalar2=0x2400,
                        op0=alu.max, op1=alu.add,
                    )
                    if nr:
                        nc.vector.reciprocal(
                            out=rr_sb[(i + 1) % 2][:, :],
                            in_=sc_sb[(i + 1) % 4][:, :],
                        ).then_inc(sRR, 2)
                    if i + 2 < NTL:
                        wait_x(eng, i + 2)
                        if i + 2 >= 4:
                            eng.wait_ge(sDU, 32 * (i - 1))   # sc buf (x4)
                        nc.vector.tensor_reduce(
                            out=amax_sb[i % 2][:, :], in_=x_sb[(i + 2) % 3][:, :, :],
                            axis=mybir.AxisListType.X, op=alu.max,
                            apply_absolute_value=True,
                        )
                    nc.vector.tensor_scalar(
                        out=s_i32[:, :], in0=xs_i32[i % 2][:, :],
                        scalar1=-2147450880, scalar2=None, op0=alu.bitwise_and,
                    )
                    nc.vector.tensor_tensor(
                        out=t_i32[:, :], in0=t_i32[:, :],
                        in1=s_i32[:, :], op=alu.bitwise_or,
                    )
                    nc.vector.tensor_tensor(
                        out=y_sb[i % 2][:, :], in0=xs_sb[i % 2][:, :],
                        in1=t_sb[:, :], op=alu.add,
                    )
                    nc.vector.tensor_tensor(
                        out=y_sb[i % 2][:, :], in0=y_sb[i % 2][:, :],
                        in1=t_sb[:, :], op=alu.subtract,
                    ).then_inc(sFR, 2)

                stats(0, half=0)
                stats(0, half=1)
                front(0, half=0)
                stats(1)
                front(0, half=1)
                stats(2)
                front(1)
                stats(3, half=0)
                stats(3, half=1)
                # front(2) + trailing reduce(4) (sc/recip(4) happen in chain(3))
                front(2)
                wait_x(eng, 4)
                eng.wait_ge(sDU, 32)
                nc.vector.tensor_reduce(
                    out=amax_sb[0][:, :], in_=x_sb[1][:, :, :],
                    axis=mybir.AxisListType.X, op=alu.max,
                    apply_absolute_value=True,
                )
                for i in range(3, NTL):
                    chain(i)

            @block.gpsimd
            def _(eng):
                def xs_op(i, gl, gh):
                    r_b = rr_sb[i % 2][:, gl:gh].unsqueeze(2).broadcast_to(
                        (TT, gh - gl, GS)
                    )
                    nc.gpsimd.tensor_tensor(
                        out=xs_sb[i % 2].rearrange(
                            "p (g s) -> p g s", s=GS)[:, gl:gh, :],
                        in0=x_sb[i % 3][:, gl:gh, :],
                        in1=r_b, op=alu.mult,
                    )

                # tile0 in halves (fill latency)
                eng.wait_ge(sX0A, 16)
                eng.wait_ge(sRR, 1)
                xs_op(0, 0, G // 2)
                nc.gpsimd.drain().then_inc(sXS, 1)
                eng.wait_ge(sX0B, 16)
                eng.wait_ge(sRR, 2)
                xs_op(0, G // 2, G)
                nc.gpsimd.drain().then_inc(sXS, 1)
                for i in range(1, NTL):
                    wait_x(eng, i)
                    eng.wait_ge(sRR, 2 * (i + 1))
                    if i >= 2:
                        eng.wait_ge(sFR, 2 * (i - 1))  # xs buf free
                    xs_op(i, 0, G)
                    nc.gpsimd.drain().then_inc(sXS, 2)

            @block.scalar
            def _(eng):
                def mk_dq(j):
                    if j >= NTL:
                        return
                    # tile0's front is split in halves (sFR +1 each); later
                    # fronts are whole (+2) -- wait only on exact sem values
                    eng.wait_ge(sFR, 1 if j == 0 else 2 * (j + 1))
                    if j >= 2:
                        wait_tp(eng, j - 2, full=True)    # dq buf free
                    for g in range(G):
                        if g == 16 and j == 0:
                            eng.wait_ge(sFR, 2)
                        nc.scalar.activation(
                            out=dq3[j % 2][:, g, :], in_=y3[j % 2][:, g, :],
                            func=AF.Copy, scale=sc_sb[j % 4][:, g:g + 1],
                        )
                        if g % 16 == 15:
                            nc.scalar.drain().then_inc(sDU, 16)
                            if j > 0:
                                stp = sTPa if g == 15 else sTPb
                                if g == 15 and j >= 2:
                                    eng.wait_ge(sMM, j - 1)   # dqt buf free
                                eng.wait_ge(stp, 16 * j)      # sem-order guard
                                eng.dma_start(
                                    out=dqt_sb[j % 2][:, g - 15:g + 1, :],
                                    in_=dq_sb[j % 2][:, 128 * (g - 15):128 * (g + 1)],
                                    transpose=True,
                                ).then_inc(stp, 16)

                def out_copy(j):
                    if j < 0 or j >= NTL:
                        return
                    eng.wait_ge(sMM, j + 1)
                    sod = sODa if j % 2 == 0 else sODb
                    if j >= 2:
                        eng.wait_ge(sod, 16 * (j // 2))  # o_sb free
                    nc.scalar.activation(
                        out=o_sb[j % 2][:, :], in_=pout_ps[j % 2][:, :],
                        func=AF.Copy,
                    )
                    nc.scalar.drain().then_inc(sOC, 1)
                    eng.dma_start(
                        out=out_d[j * TT:(j + 1) * TT, :], in_=o_sb[j % 2][:, :]
                    ).then_inc(sod, 16)

                for i in range(NTL):
                    mk_dq(i)
                    out_copy(i - 2)
                out_copy(NTL - 2)
                out_copy(NTL - 1)

            @block.tensor
            def _(eng):
                # self-paced warmup: keeps PE continuously busy (and the
                # p-state ramped) until tile0's transposes land.  Reads
                # whatever is in SBUF; results go to a scratch PSUM bank.
                eng.wait_ge(sV, 1)
                for p in range(WARM):
                    if p >= 2:
                        eng.wait_ge(sWU, 2 * p - 4)
                    for _ in range(2):
                        nc.tensor.matmul(
                            warm_ps[:, :],
                            lhsT=warm_sb[:, 0:128],
                            rhs=warm_sb[:, :],
                            start=True, stop=True,
                        ).then_inc(sWU, 1)

                for i in range(NTL):
                    if i >= 2:
                        eng.wait_ge(sOC, i - 1)    # pout buf free
                    if i == 0:
                        # coarse start: everything tile0 needs is resident
                        # (the warmup stream keeps the PE busy until here)
                        eng.wait_ge(sWT, 128)
                        eng.wait_ge(sC, 16)
                        eng.wait_ge(sV, 1)
                    for b in range(G):
                        if b == 0:
                            wait_tp(eng, i, full=False)
                        elif b == 16:
                            wait_tp(eng, i, full=True)
                        for hf in range(2):
                            nc.tensor.matmul(
                                pout_ps[i % 2][:, hf * 512:(hf + 1) * 512],
                                lhsT=dqt_sb[i % 2][:, b, :],
                                rhs=wt_sb[:, b, hf * 512:(hf + 1) * 512],
                                start=(b == 0),
                                stop=False,
                            )
                    for hf in range(2):
                        ins = nc.tensor.matmul(
                            pout_ps[i % 2][:, hf * 512:(hf + 1) * 512],
                            lhsT=ones_sb[:, :],
                            rhs=bias_sb[:, hf * 512:(hf + 1) * 512],
                            start=False,
                            stop=True,
                        )
                        if hf == 1:
                            ins.then_inc(sMM, 1)

    return nc


def _get_nc():
    if "nc" not in _CACHE:
        _CACHE["nc"] = _build_bass()
    return _CACHE["nc"]


def make_in_maps(x, weight, bias):
    x2 = np.ascontiguousarray(np.asarray(x, dtype=np.float32).reshape(TOK, K))
    wt = np.ascontiguousarray(np.asarray(weight, dtype=np.float32).T).astype(
        np.float16
    )
    bias_h = np.asarray(bias, dtype=np.float32).reshape(1, M).astype(np.float16)
    return [
        {"x": x2[i * TPC:(i + 1) * TPC], "wt": wt, "bias": bias_h}
        for i in range(N_CORES)
    ]


def kernel(x, weight, bias, grid=None, **_ignored):
    from concourse.bass_utils import run_bass_kernel_spmd

    nc = _get_nc()
    in_maps = make_in_maps(x, weight, bias)
    res = run_bass_kernel_spmd(nc, in_maps, core_ids=list(range(N_CORES)))
    out = np.concatenate([res.results[i]["out"] for i in range(N_CORES)], axis=0)
    return out.reshape(4, 4096, M).astype(np.float32)


# revision 43
# speedup vs baseline: 1.0237x; 1.0237x over previous
"""ANT_Linear fused kernel for 8 TRN2 NeuronCores (raw Bass, manual sems).

out = fakequant(x) @ W.T + bias; per-128-group absmax scaling of x snapped to
the 15-level e2m1 ('flint') grid.  Data-parallel over tokens: 2048/core,
16 tiles of [128 tokens, 4096 features].

v3: PE-saturating schedule with a signed magic-add snap:

  xs = RN16(x*(6/absmax))               (Pool broadcast mult)
  t' = copysign(512*2^floor(log2(max(|xs|,1))), xs)
       via uint16-lane bit ops: exp-mask, clamp, +9<<10, or-sign  (DVE)
  c  = RN16(xs + t')                    (DVE: rounds xs at the grid quantum)
  y  = c - t'                           (DVE, exact: y = snap(xs) signed)
  dq = y * scale                        (ACT per-group Copy-with-scale)
  dqT = dma_transpose(dq); out = dqT.T @ W.T + bias on PE (fp16 matmuls)

Engine busy per [128,4096] tile (cost-model ns):
  DVE : reduce 4297 + sc/rr 171 + t_e/t/s 3x1127 + t' 2163 + c/y 2x2194
        -> ~14.7us/tile  <- pipeline bound
  PE  : 64 fp16 matmuls + 2 bias matmuls = 14.1us
  ACT : dq 32x292 + psum->sbuf out copy = 10.4us;  Pool: xs 8.3us
  DMA : x 5.8 + 2 transpose halves 3.6 + out 1.5 = 10.9us (single queue)

Schedule notes:
 - A self-paced warmup stream of dummy matmuls (pairs, self-incrementing
   sem, 2-pair lookahead) keeps the PE engine continuously busy -- and the
   cost model's p-state ramp warm -- until tile0's inputs are resident
   (~51us: the FIFO DMA queue must carry x0..x3 + all weights + bias +
   tile0's transposes first).  Tile0 then starts coarse (all waits up
   front) and the stream stays gap-free; the quant chain runs ~1.5 tiles
   ahead of the PE throughout.
 - tile0's x-load/stats/xs/front/dq are split in halves to shorten the
   fill; the DMA issue order is hand-choreographed (x2 gated on rr0, the
   last 4 weight chunks gated on tile0's transposes, x4 gated on dq(1))
   because the cost model serializes all DMAs on one engine FIFO.
 - DMA completions are NOT ordered across engines: every wait must
   identify one specific DMA.  Hence per-buffer-slot x sems, per-half
   sems for the split x0/x3, h0/h1 transpose sems (sTPa/sTPb), out-DMA
   parity sems (sODa/sODb), and predecessor waits before reusing a sem.
 - Iterative DVE ops (tensor_reduce, reciprocal) need an explicit
   drain() before a dependent consumer (HW RAW hazard -- verified: NaNs
   without).  Simple ALU chains (ts/tt) are safe drain-free, and sem
   incs ride on the last compute op instead of a trailing drain.

Numerics: bit-exact with the v2 baseline except the PO2 magic constant
fixes v2's snap at |xs| in {1.997..1.999, 3.994..3.998} (v2 rounded these
to 1.5/3.0 instead of 2.0/4.0) -- rel err improves 1.43e-2 -> 1.11e-2.
"""

import numpy as np

N_CORES = 8
TOK = 4 * 4096
TPC = TOK // N_CORES    # 2048
K = 4096
M = 1024
GS = 128
G = K // GS             # 32
TT = 128
NT = TPC // TT          # 16

WARM = 112              # warmup matmul pairs

_CACHE = {}


def _register_snap_ops():
    """Register two fused custom-DVE ops implementing the signed magic-add
    snap in one pass each (f32 bit tricks inside the DVE pipe):
      ANT_SNAP_C: c = RN16(xs + t'(xs))
      ANT_SNAP_Y: y = c - t'(xs)        (sign taken from c; sign(c)==sign(xs))
    with t'(u) = copysign(512 * 2^floor(log2(max(|u|,1))), u).
    """
    if "snap_ops" in _CACHE:
        return _CACHE["snap_ops"]
    import concourse.dve_ops as dops
    from concourse.dve_spec import (AluOp, Bin, C0, C1, C2, Spec, Src0, Src1,
                                    Zero, _has_src1, lower, maxx, select)
    from concourse.dve_uop import DveOpSpec

    def _tmag_np(src, s1, imm2):
        b = (src.astype(np.float32).view(np.int32) & 0x7F800000).view(
            np.float32)
        return np.maximum(b, np.float32(s1)) * np.float32(imm2)

    def ref_snap_c(in0, in1, s0, s1, imm2):
        x = in0.astype(np.float32)
        t = _tmag_np(x, s1, imm2)
        return x + np.where(x < 0, -t, t).astype(np.float32)

    def ref_snap_y(in0, in1, s0, s1, imm2):
        c = in0.astype(np.float32)
        t = _tmag_np(in1, s1, imm2)
        return c - np.where(c < 0, -t, t).astype(np.float32)

    _tm0 = maxx(Bin(AluOp.BITWISE_AND, Src0, C0), C1) * C2
    spec_c = Spec(body=Src0 + select(Src0 < Zero, Zero - _tm0, _tm0),
                  reference=ref_snap_c)
    _tm1 = maxx(Bin(AluOp.BITWISE_AND, Src1, C0), C1) * C2
    spec_y = Spec(body=Src0 - select(Src0 < Zero, Zero - _tm1, _tm1),
                  reference=ref_snap_y)

    ops = []
    for name, spec in (("ANT_SNAP_C", spec_c), ("ANT_SNAP_Y", spec_y)):
        if name in dops._SUB_OPCODE_FOR_NAME:
            ops.append(next(o for o in dops.OPS if o.name == name))
            continue
        row = max(dops._SUB_OPCODE_FOR_NAME.values()) + 1
        assert row < 0x20
        dops._SUB_OPCODE_FOR_NAME[name] = row
        shas = {}
        for ver in ("v3", "v4"):
            uops = lower(spec, ver=ver)
            sp = DveOpSpec(name=name, opcode=row, uops=uops,
                           rd1_en=_has_src1(spec))
            shas[ver] = sp.sha(ver)
        op = dops.DveOp(name, spec, subdim=False, uops_sha=shas)
        dops.OPS.append(op)
        dops.CUSTOM_DVE_SPECS[name] = spec
        ops.append(op)
    _CACHE["snap_ops"] = tuple(ops)
    return _CACHE["snap_ops"]


def _build_bass(nt=NT):
    from contextlib import ExitStack

    import concourse.bass as bass
    import concourse.mybir as mybir

    dt = mybir.dt
    alu = mybir.AluOpType
    AF = mybir.ActivationFunctionType

    SNAP_C, SNAP_Y = _register_snap_ops()

    NTL = nt
    nc = bass.Bass()
    x_d = nc.declare_dram_parameter("x", [NTL * TT, K], dt.float32, isOutput=False)
    wt_d = nc.declare_dram_parameter("wt", [K, M], dt.float16, isOutput=False)
    b_d = nc.declare_dram_parameter("bias", [1, M], dt.float16, isOutput=False)
    out_d = nc.declare_dram_parameter("out", [NTL * TT, M], dt.float32, isOutput=True)

    x_t4 = x_d.rearrange("(n p) (g s) -> n p g s", p=TT, s=GS)   # [16,128,32,128]
    wt_t3 = wt_d.rearrange("(b p) m -> p b m", p=128)            # [128,32,1024]

    ctx = ExitStack()
    with ctx:
        sb = lambda name, shape, d: ctx.enter_context(nc.sbuf_tensor(name, shape, d))
        ps = lambda name, shape, d: ctx.enter_context(nc.psum_tensor(name, shape, d))
        sem = lambda name: ctx.enter_context(nc.semaphore(name))

        wt_sb = sb("wt_sb", [128, G, M], dt.float16)            # 8 MiB resident
        bias_sb = sb("bias_sb", [1, M], dt.float16)
        ones_sb = sb("ones_sb", [1, TT], dt.float16)

        x_sb = [sb(f"x_sb{k}", [TT, G, GS], dt.float32) for k in range(3)]
        xs_sb = [sb(f"xs_sb{k}", [TT, K], dt.float16) for k in range(2)]
        t_sb = sb("t_sb", [TT, K], dt.float16)
        s_sb = sb("s_sb", [TT, K], dt.float16)
        y_sb = [sb(f"y_sb{k}", [TT, K], dt.float16) for k in range(2)]
        dq_sb = [sb(f"dq_sb{k}", [TT, K], dt.float16) for k in range(2)]
        dqt_sb = [sb(f"dqt_sb{k}", [128, G, TT], dt.float16) for k in range(2)]
        o_sb = [sb(f"o_sb{k}", [TT, M], dt.float32) for k in range(2)]
        warm_sb = sb("warm_sb", [TT, 512], dt.float16)
        amax_sb = [sb(f"amax_sb{k}", [TT, G], dt.float32) for k in range(2)]
        sc_sb = [sb(f"sc_sb{k}", [TT, G], dt.float32) for k in range(4)]
        rr_sb = [sb(f"rr_sb{k}", [TT, G], dt.float32) for k in range(2)]

        pout_ps = [ps(f"pout_ps{k}", [TT, M], dt.float32) for k in range(2)]
        warm_ps = ps("warm_ps", [TT, 512], dt.float32)

        sC = sem("sC")     # bias DMA done (+16)
        sV = sem("sV")     # ones/warm memset done
        # x-load sems: one per buffer slot (+ dedicated sems for the four
        # half-loads of x0/x3) so a sem value always identifies ONE DMA --
        # DMA completions are not ordered across engines.
        sXA = sem("sXA")   # slot0 whole loads: x6, x9, x12, x15
        sXB = sem("sXB")   # slot1 loads: x1, x4, x7, x10, x13
        sXC = sem("sXC")   # slot2 loads: x2, x5, x8, x11, x14
        sX0A = sem("sX0A")  # x0 first half
        sX0B = sem("sX0B")  # x0 second half
        sX3A = sem("sX3A")  # x3 first half
        sX3B = sem("sX3B")  # x3 second half
        sWT = sem("sWT")   # weight chunk in (+16/chunk)
        sRR = sem("sRR")   # stats done (amax/sc/rr) (+1/half for tile0, +2/tile)
        sXS = sem("sXS")   # Pool xs done (+1/half tile0, +2/tile)
        sFR = sem("sFR")   # DVE front chain done (+1/half tile0, +2/tile)
        sDU = sem("sDU")   # ACT dq half done (+16/half)
        sTPa = sem("sTPa")  # transpose h0 done (+16/tile)
        sTPb = sem("sTPb")  # transpose h1 done (+16/tile)
        sMM = sem("sMM")   # PE tile done (+1/tile)
        sOC = sem("sOC")   # ACT out copy done (+1/tile)
        sODa = sem("sODa")  # out DMA done, even tiles (+16)
        sODb = sem("sODb")  # out DMA done, odd tiles (+16)
        sWU = sem("sWU")   # warmup pacing (+1/warmup mm)

        def x_sem(i):
            """(sem, value) identifying completion of whole-load x(i)."""
            if i % 3 == 0:
                return sXA, 16 * (i // 3 - 1)
            if i % 3 == 1:
                return sXB, 16 * ((i - 1) // 3 + 1)
            return sXC, 16 * ((i - 2) // 3 + 1)

        def wait_x(eng, j):
            """Wait for x(j) fully loaded."""
            if j == 0:
                eng.wait_ge(sX0A, 16)
                eng.wait_ge(sX0B, 16)
            elif j == 3:
                eng.wait_ge(sX3A, 16)
                eng.wait_ge(sX3B, 16)
            else:
                s, v = x_sem(j)
                eng.wait_ge(s, v)

        xs_i32 = [xs_sb[k].bitcast(dt.int32) for k in range(2)]
        xs_u16 = [xs_sb[k].bitcast(dt.uint16) for k in range(2)]
        t_i32 = t_sb.bitcast(dt.int32)
        t_u16 = t_sb.bitcast(dt.uint16)
        s_i32 = s_sb.bitcast(dt.int32)
        y3 = [y_sb[k].rearrange("p (g s) -> p g s", s=GS) for k in range(2)]
        dq3 = [dq_sb[k].rearrange("p (g s) -> p g s", s=GS) for k in range(2)]

        def wait_tp(eng, j, full):
            """Wait for tile j's transpose h0 (full=False) or both halves."""
            eng.wait_ge(sTPa, 16 * (j + 1))
            if full:
                eng.wait_ge(sTPb, 16 * (j + 1))

        with nc.Block() as block:

            @block.sync
            def _(eng):
                # FIFO DMA queue choreography (see docstring).
                eng.dma_start(
                    out=x_sb[0][:, 0:16, :], in_=x_t4[0][:, 0:16, :]
                ).then_inc(sX0A, 16)
                eng.dma_start(
                    out=x_sb[0][:, 16:32, :], in_=x_t4[0][:, 16:32, :]
                ).then_inc(sX0B, 16)
                eng.dma_start(out=x_sb[1][:, :, :], in_=x_t4[1]).then_inc(sXB, 16)
                eng.dma_start(out=bias_sb[:, :], in_=b_d[:, :]).then_inc(sC, 16)
                for c in range(2):
                    eng.dma_start(
                        out=wt_sb[:, 4 * c:4 * (c + 1), :],
                        in_=wt_t3[:, 4 * c:4 * (c + 1), :],
                    ).then_inc(sWT, 16)
                eng.wait_ge(sRR, 2)                # x2 right when stats need it
                eng.dma_start(out=x_sb[2][:, :, :], in_=x_t4[2]).then_inc(sXC, 16)
                for c in range(2, 6):
                    eng.dma_start(
                        out=wt_sb[:, 4 * c:4 * (c + 1), :],
                        in_=wt_t3[:, 4 * c:4 * (c + 1), :],
                    ).then_inc(sWT, 16)
                eng.wait_ge(sDU, 16)               # tile0 dq half ready
                eng.dma_start(
                    out=dqt_sb[0][:, 0:16, :], in_=dq_sb[0][:, 0:2048],
                    transpose=True,
                ).then_inc(sTPa, 16)
                eng.wait_ge(sDU, 32)
                eng.dma_start(
                    out=dqt_sb[0][:, 16:32, :], in_=dq_sb[0][:, 2048:4096],
                    transpose=True,
                ).then_inc(sTPb, 16)
                eng.dma_start(
                    out=x_sb[0][:, 0:16, :], in_=x_t4[3][:, 0:16, :]
                ).then_inc(sX3A, 16)
                for c in range(6, 8):
                    eng.dma_start(
                        out=wt_sb[:, 4 * c:4 * (c + 1), :],
                        in_=wt_t3[:, 4 * c:4 * (c + 1), :],
                    ).then_inc(sWT, 16)
                eng.dma_start(
                    out=x_sb[0][:, 16:32, :], in_=x_t4[3][:, 16:32, :]
                ).then_inc(sX3B, 16)
                for i in range(4, NTL):
                    if i == 4:
                        # don't head-of-line block tile1's transposes
                        eng.wait_ge(sDU, 64)
                    eng.wait_ge(sXS, 2 * (i - 2))  # x buf free (x3 ring)
                    s, v = x_sem(i)
                    if v > 16:
                        eng.wait_ge(s, v - 16)     # slot predecessor done
                    eng.dma_start(
                        out=x_sb[i % 3][:, :, :], in_=x_t4[i]
                    ).then_inc(s, 16)

            @block.vector
            def _(eng):
                nc.vector.memset(ones_sb[:, :], 1.0)
                nc.vector.memset(warm_sb[:, :], 0.5)
                nc.vector.drain().then_inc(sV, 1)

                def stats(j, half=None):
                    if j >= NTL:
                        return
                    gl, gh = (0, G) if half is None else (
                        (0, G // 2) if half == 0 else (G // 2, G))
                    if j == 0:
                        eng.wait_ge(sX0A if half == 0 else sX0B, 16)
                    elif j == 3:
                        eng.wait_ge(sX3A if half == 0 else sX3B, 16)
                    else:
                        wait_x(eng, j)
                    if j >= 4:
                        # sc buf (x4): dq(j-4) groups all scaled
                        eng.wait_ge(sDU, 32 * (j - 3))
                    nc.vector.tensor_reduce(
                        out=amax_sb[j % 2][:, gl:gh],
                        in_=x_sb[j % 3][:, gl:gh, :],
                        axis=mybir.AxisListType.X, op=alu.max,
                        apply_absolute_value=True,
                    )
                    nc.vector.drain()
                    # scale = max(amax/6, tiny); rr = 1/scale
                    nc.vector.tensor_scalar(
                        out=sc_sb[j % 4][:, gl:gh], in0=amax_sb[j % 2][:, gl:gh],
                        scalar1=1.0 / 6.0, scalar2=1e-30,
                        op0=alu.mult, op1=alu.max,
                    )
                    nc.vector.drain()
                    nc.vector.reciprocal(
                        out=rr_sb[j % 2][:, gl:gh], in_=sc_sb[j % 4][:, gl:gh]
                    ).then_inc(sRR, 2 if half is None else 1)

                def front(j, half=None):
                    if j < 0 or j >= NTL:
                        return
                    lo, hi = (0, K) if half is None else (
                        (0, K // 2) if half == 0 else (K // 2, K))
                    eng.wait_ge(sXS, 2 * j + 1 + (half != 0))
                    if j >= 2:
                        eng.wait_ge(sDU, 32 * (j - 1))  # y buf free
                    l2, h2 = lo // 2, hi // 2
                    # t_e = exp_bits(xs)
                    nc.vector.tensor_scalar(
                        out=t_u16[:, lo:hi], in0=xs_u16[j % 2][:, lo:hi],
                        scalar1=0x7C00, scalar2=None, op0=alu.bitwise_and,
                    )
                    # t = (max(t_e, bits(1.0)) + 0x2400: PO2 of max(|xs|,1)
                    # times 512, as an exponent-field add
                    nc.vector.tensor_scalar(
                        out=t_u16[:, lo:hi], in0=t_u16[:, lo:hi],
                        scalar1=0x3C00, scalar2=0x2400,
                        op0=alu.max, op1=alu.add,
                    )
                    # s = sign pairs
                    nc.vector.tensor_scalar(
                        out=s_i32[:, l2:h2], in0=xs_i32[j % 2][:, l2:h2],
                        scalar1=-2147450880, scalar2=None, op0=alu.bitwise_and,
                    )
                    # t' = t | s  (copysign, in place over t)
                    nc.vector.tensor_tensor(
                        out=t_i32[:, l2:h2], in0=t_i32[:, l2:h2],
                        in1=s_i32[:, l2:h2], op=alu.bitwise_or,
                    )
                    # c = RN16(xs + t'): rounds xs at the grid quantum
                    nc.vector.tensor_tensor(
                        out=y_sb[j % 2][:, lo:hi], in0=xs_sb[j % 2][:, lo:hi],
                        in1=t_sb[:, lo:hi], op=alu.add,
                    )
                    # y = c - t'  (exact; signed snap)
                    nc.vector.tensor_tensor(
                        out=y_sb[j % 2][:, lo:hi], in0=y_sb[j % 2][:, lo:hi],
                        in1=t_sb[:, lo:hi], op=alu.subtract,
                    ).then_inc(sFR, 2 if half is None else 1)

                stats(0, half=0)
                stats(0, half=1)
                front(0, half=0)
                stats(1)
                front(0, half=1)
                stats(2)
                for i in range(1, NTL):
                    front(i)
                    if i + 2 == 3:
                        stats(3, half=0)
                        stats(3, half=1)
                    else:
                        stats(i + 2)

            @block.gpsimd
            def _(eng):
                def xs_op(i, gl, gh):
                    r_b = rr_sb[i % 2][:, gl:gh].unsqueeze(2).broadcast_to(
                        (TT, gh - gl, GS)
                    )
                    nc.gpsimd.tensor_tensor(
                        out=xs_sb[i % 2].rearrange(
                            "p (g s) -> p g s", s=GS)[:, gl:gh, :],
                        in0=x_sb[i % 3][:, gl:gh, :],
                        in1=r_b, op=alu.mult,
                    )

                # tile0 in halves (fill latency)
                eng.wait_ge(sX0A, 16)
                eng.wait_ge(sRR, 1)
                xs_op(0, 0, G // 2)
                nc.gpsimd.drain().then_inc(sXS, 1)
                eng.wait_ge(sX0B, 16)
                eng.wait_ge(sRR, 2)
                xs_op(0, G // 2, G)
                nc.gpsimd.drain().then_inc(sXS, 1)
                for i in range(1, NTL):
                    wait_x(eng, i)
                    eng.wait_ge(sRR, 2 * (i + 1))
                    if i >= 2:
                        eng.wait_ge(sFR, 2 * (i - 1))  # xs buf free
                    xs_op(i, 0, G)
                    nc.gpsimd.drain().then_inc(sXS, 2)

            @block.scalar
            def _(eng):
                def mk_dq(j):
                    if j >= NTL:
                        return
                    # tile0's front is split in halves (sFR +1 each); later
                    # fronts are whole (+2) -- wait only on exact sem values
                    eng.wait_ge(sFR, 1 if j == 0 else 2 * (j + 1))
                    if j >= 2:
                        wait_tp(eng, j - 2, full=True)    # dq buf free
                    for g in range(G):
                        if g == 16 and j == 0:
                            eng.wait_ge(sFR, 2)
                        nc.scalar.activation(
                            out=dq3[j % 2][:, g, :], in_=y3[j % 2][:, g, :],
                            func=AF.Copy, scale=sc_sb[j % 4][:, g:g + 1],
                        )
                        if g % 16 == 15:
                            nc.scalar.drain().then_inc(sDU, 16)
                            if j > 0:
                                stp = sTPa if g == 15 else sTPb
                                if g == 15 and j >= 2:
                                    eng.wait_ge(sMM, j - 1)   # dqt buf free
                                eng.wait_ge(stp, 16 * j)      # sem-order guard
                                eng.dma_start(
                                    out=dqt_sb[j % 2][:, g - 15:g + 1, :],
                                    in_=dq_sb[j % 2][:, 128 * (g - 15):128 * (g + 1)],
                                    transpose=True,
                                ).then_inc(stp, 16)

                def out_copy(j):
                    if j < 0 or j >= NTL:
                        return
                    eng.wait_ge(sMM, j + 1)
                    sod = sODa if j % 2 == 0 else sODb
                    if j >= 2:
                        eng.wait_ge(sod, 16 * (j // 2))  # o_sb free
                    nc.scalar.activation(
                        out=o_sb[j % 2][:, :], in_=pout_ps[j % 2][:, :],
                        func=AF.Copy,
                    )
                    nc.scalar.drain().then_inc(sOC, 1)
                    eng.dma_start(
                        out=out_d[j * TT:(j + 1) * TT, :], in_=o_sb[j % 2][:, :]
                    ).then_inc(sod, 16)

                for i in range(NTL):
                    mk_dq(i)
                    out_copy(i - 2)
                out_copy(NTL - 2)
                out_copy(NTL - 1)

            @block.tensor
            def _(eng):
                # self-paced warmup: keeps PE continuously busy (and the
                # p-state ramped) until tile0's transposes land.  Reads
                # whatever is in SBUF; results go to a scratch PSUM bank.
                eng.wait_ge(sV, 1)
                for p in range(WARM):
                    if p >= 2:
                        eng.wait_ge(sWU, 2 * p - 4)
                    for _ in range(2):
                        nc.tensor.matmul(
                            warm_ps[:, :],
                            lhsT=warm_sb[:, 0:128],
                            rhs=warm_sb[:, :],
                            start=True, stop=True,
                        ).then_inc(sWU, 1)

                for i in range(NTL):
                    if i >= 2:
                        eng.wait_ge(sOC, i - 1)    # pout buf free
                    if i == 0:
                        # coarse start: everything tile0 needs is resident
                        # (the warmup stream keeps the PE busy until here)
                        eng.wait_ge(sWT, 128)
                        eng.wait_ge(sC, 16)
                        eng.wait_ge(sV, 1)
                    for b in range(G):
                        if b == 0:
                            wait_tp(eng, i, full=False)
                        elif b == 16:
                            wait_tp(eng, i, full=True)
                        for hf in range(2):
                            nc.tensor.matmul(
                                pout_ps[i % 2][:, hf * 512:(hf + 1) * 512],
                                lhsT=dqt_sb[i % 2][:, b, :],
                                rhs=wt_sb[:, b, hf * 512:(hf + 1) * 512],
                                start=(b == 0),
                                stop=False,
                            )
                    for hf in range(2):
                        ins = nc.tensor.matmul(
                            pout_ps[i % 2][:, hf * 512:(hf + 1) * 512],
                            lhsT=ones_sb[:, :],
                            rhs=bias_sb[:, hf * 512:(hf + 1) * 512],
                            start=False,
                            stop=True,
                        )
                        if hf == 1:
                            ins.then_inc(sMM, 1)

    return nc


def _get_nc():
    if "nc" not in _CACHE:
        _CACHE["nc"] = _build_bass()
    return _CACHE["nc"]


def make_in_maps(x, weight, bias):
    x2 = np.ascontiguousarray(np.asarray(x, dtype=np.float32).reshape(TOK, K))
    wt = np.ascontiguousarray(np.asarray(weight, dtype=np.float32).T).astype(
        np.float16
    )
    bias_h = np.asarray(bias, dtype=np.float32).reshape(1, M).astype(np.float16)
    return [
        {"x": x2[i * TPC:(i + 1) * TPC], "wt": wt, "bias": bias_h}
        for i in range(N_CORES)
    ]


def kernel(x, weight, bias, grid=None, **_ignored):
    from concourse.bass_utils import run_bass_kernel_spmd

    nc = _get_nc()
    in_maps = make_in_maps(x, weight, bias)
    res = run_bass_kernel_spmd(nc, in_maps, core_ids=list(range(N_CORES)))
    out = np.concatenate([res.results[i]["out"] for i in range(N_CORES)], axis=0)
    return out.reshape(4, 4096, M).astype(np.float32)


# revision 44
# speedup vs baseline: 1.0260x; 1.0022x over previous
"""ANT_Linear fused kernel for 8 TRN2 NeuronCores (raw Bass, manual sems).

out = fakequant(x) @ W.T + bias; per-128-group absmax scaling of x snapped to
the 15-level e2m1 ('flint') grid.  Data-parallel over tokens: 2048/core,
16 tiles of [128 tokens, 4096 features].

v3: PE-saturating schedule with a signed magic-add snap:

  xs = RN16(x*(6/absmax))               (Pool broadcast mult)
  t' = copysign(512*2^floor(log2(max(|xs|,1))), xs)
       via uint16-lane bit ops: exp-mask, clamp, +9<<10, or-sign  (DVE)
  c  = RN16(xs + t')                    (DVE: rounds xs at the grid quantum)
  y  = c - t'                           (DVE, exact: y = snap(xs) signed)
  dq = y * scale                        (ACT per-group Copy-with-scale)
  dqT = dma_transpose(dq); out = dqT.T @ W.T + bias on PE (fp16 matmuls)

Engine busy per [128,4096] tile (cost-model ns):
  DVE : reduce 4297 + sc/rr 171 + t_e/t/s 3x1127 + t' 2163 + c/y 2x2194
        -> ~14.7us/tile  <- pipeline bound
  PE  : 64 fp16 matmuls + 2 bias matmuls = 14.1us
  ACT : dq 32x292 + psum->sbuf out copy = 10.4us;  Pool: xs 8.3us
  DMA : x 5.8 + 2 transpose halves 3.6 + out 1.5 = 10.9us (single queue)

Schedule notes:
 - A self-paced warmup stream of dummy matmuls (pairs, self-incrementing
   sem, 2-pair lookahead) keeps the PE engine continuously busy -- and the
   cost model's p-state ramp warm -- until tile0's inputs are resident
   (~51us: the FIFO DMA queue must carry x0..x3 + all weights + bias +
   tile0's transposes first).  Tile0 then starts coarse (all waits up
   front) and the stream stays gap-free; the quant chain runs ~1.5 tiles
   ahead of the PE throughout.
 - tile0's x-load/stats/xs/front/dq are split in halves to shorten the
   fill; the DMA issue order is hand-choreographed (x2 gated on rr0, the
   last 4 weight chunks gated on tile0's transposes, x4 gated on dq(1))
   because the cost model serializes all DMAs on one engine FIFO.
 - DMA completions are NOT ordered across engines: every wait must
   identify one specific DMA.  Hence per-buffer-slot x sems, per-half
   sems for the split x0/x3, h0/h1 transpose sems (sTPa/sTPb), out-DMA
   parity sems (sODa/sODb), and predecessor waits before reusing a sem.
 - Iterative DVE ops (tensor_reduce, reciprocal) need an explicit
   drain() before a dependent consumer (HW RAW hazard -- verified: NaNs
   without).  Simple ALU chains (ts/tt) are safe drain-free, and sem
   incs ride on the last compute op instead of a trailing drain.

Numerics: bit-exact with the v2 baseline except the PO2 magic constant
fixes v2's snap at |xs| in {1.997..1.999, 3.994..3.998} (v2 rounded these
to 1.5/3.0 instead of 2.0/4.0) -- rel err improves 1.43e-2 -> 1.11e-2.
"""

import numpy as np

N_CORES = 8
TOK = 4 * 4096
TPC = TOK // N_CORES    # 2048
K = 4096
M = 1024
GS = 128
G = K // GS             # 32
TT = 128
NT = TPC // TT          # 16

WARM = 112              # warmup matmul pairs

_CACHE = {}


def _register_snap_ops():
    """Register two fused custom-DVE ops implementing the signed magic-add
    snap in one pass each (f32 bit tricks inside the DVE pipe):
      ANT_SNAP_C: c = RN16(xs + t'(xs))
      ANT_SNAP_Y: y = c - t'(xs)        (sign taken from c; sign(c)==sign(xs))
    with t'(u) = copysign(512 * 2^floor(log2(max(|u|,1))), u).
    """
    if "snap_ops" in _CACHE:
        return _CACHE["snap_ops"]
    import concourse.dve_ops as dops
    from concourse.dve_spec import (AluOp, Bin, C0, C1, C2, Spec, Src0, Src1,
                                    Zero, _has_src1, lower, maxx, select)
    from concourse.dve_uop import DveOpSpec

    def _tmag_np(src, s1, imm2):
        b = (src.astype(np.float32).view(np.int32) & 0x7F800000).view(
            np.float32)
        return np.maximum(b, np.float32(s1)) * np.float32(imm2)

    def ref_snap_c(in0, in1, s0, s1, imm2):
        x = in0.astype(np.float32)
        t = _tmag_np(x, s1, imm2)
        return x + np.where(x < 0, -t, t).astype(np.float32)

    def ref_snap_y(in0, in1, s0, s1, imm2):
        c = in0.astype(np.float32)
        t = _tmag_np(in1, s1, imm2)
        return c - np.where(c < 0, -t, t).astype(np.float32)

    _tm0 = maxx(Bin(AluOp.BITWISE_AND, Src0, C0), C1) * C2
    spec_c = Spec(body=Src0 + select(Src0 < Zero, Zero - _tm0, _tm0),
                  reference=ref_snap_c)
    _tm1 = maxx(Bin(AluOp.BITWISE_AND, Src1, C0), C1) * C2
    spec_y = Spec(body=Src0 - select(Src0 < Zero, Zero - _tm1, _tm1),
                  reference=ref_snap_y)

    ops = []
    for name, spec in (("ANT_SNAP_C", spec_c), ("ANT_SNAP_Y", spec_y)):
        if name in dops._SUB_OPCODE_FOR_NAME:
            ops.append(next(o for o in dops.OPS if o.name == name))
            continue
        row = max(dops._SUB_OPCODE_FOR_NAME.values()) + 1
        assert row < 0x20
        dops._SUB_OPCODE_FOR_NAME[name] = row
        shas = {}
        for ver in ("v3", "v4"):
            uops = lower(spec, ver=ver)
            sp = DveOpSpec(name=name, opcode=row, uops=uops,
                           rd1_en=_has_src1(spec))
            shas[ver] = sp.sha(ver)
        op = dops.DveOp(name, spec, subdim=False, uops_sha=shas)
        dops.OPS.append(op)
        dops.CUSTOM_DVE_SPECS[name] = spec
        ops.append(op)
    _CACHE["snap_ops"] = tuple(ops)
    return _CACHE["snap_ops"]


def _build_bass(nt=NT):
    from contextlib import ExitStack

    import concourse.bass as bass
    import concourse.mybir as mybir

    dt = mybir.dt
    alu = mybir.AluOpType
    AF = mybir.ActivationFunctionType

    SNAP_C, SNAP_Y = _register_snap_ops()

    NTL = nt
    nc = bass.Bass()
    x_d = nc.declare_dram_parameter("x", [NTL * TT, K], dt.float32, isOutput=False)
    wt_d = nc.declare_dram_parameter("wt", [K, M], dt.float16, isOutput=False)
    b_d = nc.declare_dram_parameter("bias", [1, M], dt.float16, isOutput=False)
    out_d = nc.declare_dram_parameter("out", [NTL * TT, M], dt.float32, isOutput=True)

    x_t4 = x_d.rearrange("(n p) (g s) -> n p g s", p=TT, s=GS)   # [16,128,32,128]
    wt_t3 = wt_d.rearrange("(b p) m -> p b m", p=128)            # [128,32,1024]

    ctx = ExitStack()
    with ctx:
        sb = lambda name, shape, d: ctx.enter_context(nc.sbuf_tensor(name, shape, d))
        ps = lambda name, shape, d: ctx.enter_context(nc.psum_tensor(name, shape, d))
        sem = lambda name: ctx.enter_context(nc.semaphore(name))

        wt_sb = sb("wt_sb", [128, G, M], dt.float16)            # 8 MiB resident
        bias_sb = sb("bias_sb", [1, M], dt.float16)
        ones_sb = sb("ones_sb", [1, TT], dt.float16)

        x_sb = [sb(f"x_sb{k}", [TT, G, GS], dt.float32) for k in range(3)]
        xs_sb = [sb(f"xs_sb{k}", [TT, K], dt.float16) for k in range(2)]
        t_sb = sb("t_sb", [TT, K], dt.float16)
        s_sb = sb("s_sb", [TT, K], dt.float16)
        y_sb = [sb(f"y_sb{k}", [TT, K], dt.float16) for k in range(2)]
        dq_sb = [sb(f"dq_sb{k}", [TT, K], dt.float16) for k in range(2)]
        dqt_sb = [sb(f"dqt_sb{k}", [128, G, TT], dt.float16) for k in range(2)]
        o_sb = [sb(f"o_sb{k}", [TT, M], dt.float32) for k in range(2)]
        warm_sb = sb("warm_sb", [TT, 512], dt.float16)
        amax_sb = [sb(f"amax_sb{k}", [TT, G], dt.float32) for k in range(2)]
        sc_sb = [sb(f"sc_sb{k}", [TT, G], dt.float32) for k in range(4)]
        rr_sb = [sb(f"rr_sb{k}", [TT, G], dt.float32) for k in range(2)]

        pout_ps = [ps(f"pout_ps{k}", [TT, M], dt.float32) for k in range(2)]
        warm_ps = ps("warm_ps", [TT, 512], dt.float32)

        sC = sem("sC")     # bias DMA done (+16)
        sV = sem("sV")     # ones/warm memset done
        # x-load sems: one per buffer slot (+ dedicated sems for the four
        # half-loads of x0/x3) so a sem value always identifies ONE DMA --
        # DMA completions are not ordered across engines.
        sXA = sem("sXA")   # slot0 whole loads: x6, x9, x12, x15
        sXB = sem("sXB")   # slot1 loads: x1, x4, x7, x10, x13
        sXC = sem("sXC")   # slot2 loads: x2, x5, x8, x11, x14
        sX0A = sem("sX0A")  # x0 first half
        sX0B = sem("sX0B")  # x0 second half
        sX3A = sem("sX3A")  # x3 first half
        sX3B = sem("sX3B")  # x3 second half
        sWT = sem("sWT")   # weight chunk in (+16/chunk)
        sRR = sem("sRR")   # stats done (amax/sc/rr) (+1/half for tile0, +2/tile)
        sXS = sem("sXS")   # Pool xs done (+1/half tile0, +2/tile)
        sFR = sem("sFR")   # DVE front chain done (+1/half tile0, +2/tile)
        sDU = sem("sDU")   # ACT dq half done (+16/half)
        sTPa = sem("sTPa")  # transpose h0 done (+16/tile)
        sTPb = sem("sTPb")  # transpose h1 done (+16/tile)
        sMM = sem("sMM")   # PE tile done (+1/tile)
        sOC = sem("sOC")   # ACT out copy done (+1/tile)
        sODa = sem("sODa")  # out DMA done, even tiles (+16)
        sODb = sem("sODb")  # out DMA done, odd tiles (+16)
        sWU = sem("sWU")   # warmup pacing (+1/warmup mm)

        def x_sem(i):
            """(sem, value) identifying completion of whole-load x(i)."""
            if i % 3 == 0:
                return sXA, 16 * (i // 3 - 1)
            if i % 3 == 1:
                return sXB, 16 * ((i - 1) // 3 + 1)
            return sXC, 16 * ((i - 2) // 3 + 1)

        def wait_x(eng, j):
            """Wait for x(j) fully loaded."""
            if j == 0:
                eng.wait_ge(sX0A, 16)
                eng.wait_ge(sX0B, 16)
            elif j == 3:
                eng.wait_ge(sX3A, 16)
                eng.wait_ge(sX3B, 16)
            else:
                s, v = x_sem(j)
                eng.wait_ge(s, v)

        xs_i32 = [xs_sb[k].bitcast(dt.int32) for k in range(2)]
        xs_u16 = [xs_sb[k].bitcast(dt.uint16) for k in range(2)]
        t_i32 = t_sb.bitcast(dt.int32)
        t_u16 = t_sb.bitcast(dt.uint16)
        s_i32 = s_sb.bitcast(dt.int32)
        y3 = [y_sb[k].rearrange("p (g s) -> p g s", s=GS) for k in range(2)]
        dq3 = [dq_sb[k].rearrange("p (g s) -> p g s", s=GS) for k in range(2)]

        def wait_tp(eng, j, full):
            """Wait for tile j's transpose h0 (full=False) or both halves."""
            eng.wait_ge(sTPa, 16 * (j + 1))
            if full:
                eng.wait_ge(sTPb, 16 * (j + 1))

        with nc.Block() as block:

            @block.sync
            def _(eng):
                # FIFO DMA queue choreography (see docstring).
                eng.dma_start(
                    out=x_sb[0][:, 0:16, :], in_=x_t4[0][:, 0:16, :]
                ).then_inc(sX0A, 16)
                eng.dma_start(
                    out=x_sb[0][:, 16:32, :], in_=x_t4[0][:, 16:32, :]
                ).then_inc(sX0B, 16)
                eng.dma_start(out=x_sb[1][:, :, :], in_=x_t4[1]).then_inc(sXB, 16)
                eng.dma_start(out=bias_sb[:, :], in_=b_d[:, :]).then_inc(sC, 16)
                for c in range(2):
                    eng.dma_start(
                        out=wt_sb[:, 4 * c:4 * (c + 1), :],
                        in_=wt_t3[:, 4 * c:4 * (c + 1), :],
                    ).then_inc(sWT, 16)
                eng.wait_ge(sRR, 2)                # x2 right when stats need it
                eng.dma_start(out=x_sb[2][:, :, :], in_=x_t4[2]).then_inc(sXC, 16)
                for c in range(2, 6):
                    eng.dma_start(
                        out=wt_sb[:, 4 * c:4 * (c + 1), :],
                        in_=wt_t3[:, 4 * c:4 * (c + 1), :],
                    ).then_inc(sWT, 16)
                eng.wait_ge(sDU, 16)               # tile0 dq half ready
                eng.dma_start(
                    out=dqt_sb[0][:, 0:16, :], in_=dq_sb[0][:, 0:2048],
                    transpose=True,
                ).then_inc(sTPa, 16)
                eng.wait_ge(sDU, 32)
                eng.dma_start(
                    out=dqt_sb[0][:, 16:32, :], in_=dq_sb[0][:, 2048:4096],
                    transpose=True,
                ).then_inc(sTPb, 16)
                eng.dma_start(
                    out=x_sb[0][:, 0:16, :], in_=x_t4[3][:, 0:16, :]
                ).then_inc(sX3A, 16)
                for c in range(6, 8):
                    eng.dma_start(
                        out=wt_sb[:, 4 * c:4 * (c + 1), :],
                        in_=wt_t3[:, 4 * c:4 * (c + 1), :],
                    ).then_inc(sWT, 16)
                eng.dma_start(
                    out=x_sb[0][:, 16:32, :], in_=x_t4[3][:, 16:32, :]
                ).then_inc(sX3B, 16)
                for i in range(4, NTL):
                    if i == 4:
                        # don't head-of-line block tile1's transposes
                        eng.wait_ge(sDU, 64)
                    eng.wait_ge(sXS, 2 * (i - 2))  # x buf free (x3 ring)
                    s, v = x_sem(i)
                    if v > 16:
                        eng.wait_ge(s, v - 16)     # slot predecessor done
                    eng.dma_start(
                        out=x_sb[i % 3][:, :, :], in_=x_t4[i]
                    ).then_inc(s, 16)

            @block.vector
            def _(eng):
                nc.vector.memset(ones_sb[:, :], 1.0)
                nc.vector.memset(warm_sb[:, :], 0.5)
                nc.vector.drain().then_inc(sV, 1)

                def stats(j, half=None):
                    if j >= NTL:
                        return
                    gl, gh = (0, G) if half is None else (
                        (0, G // 2) if half == 0 else (G // 2, G))
                    if j == 0:
                        eng.wait_ge(sX0A if half == 0 else sX0B, 16)
                    elif j == 3:
                        eng.wait_ge(sX3A if half == 0 else sX3B, 16)
                    else:
                        wait_x(eng, j)
                    if j >= 4:
                        # sc buf (x4): dq(j-4) groups all scaled
                        eng.wait_ge(sDU, 32 * (j - 3))
                    nc.vector.tensor_reduce(
                        out=amax_sb[j % 2][:, gl:gh],
                        in_=x_sb[j % 3][:, gl:gh, :],
                        axis=mybir.AxisListType.X, op=alu.max,
                        apply_absolute_value=True,
                    )
                    nc.vector.drain()
                    # scale = max(amax/6, tiny); rr = 1/scale
                    nc.vector.tensor_scalar(
                        out=sc_sb[j % 4][:, gl:gh], in0=amax_sb[j % 2][:, gl:gh],
                        scalar1=1.0 / 6.0, scalar2=1e-30,
                        op0=alu.mult, op1=alu.max,
                    )
                    nc.vector.drain()
                    nc.vector.reciprocal(
                        out=rr_sb[j % 2][:, gl:gh], in_=sc_sb[j % 4][:, gl:gh]
                    ).then_inc(sRR, 2 if half is None else 1)

                def front(j, half=None):
                    if j < 0 or j >= NTL:
                        return
                    lo, hi = (0, K) if half is None else (
                        (0, K // 2) if half == 0 else (K // 2, K))
                    eng.wait_ge(sXS, 2 * j + 1 + (half != 0))
                    if j >= 2:
                        eng.wait_ge(sDU, 32 * (j - 1))  # y buf free
                    l2, h2 = lo // 2, hi // 2
                    # t_e = exp_bits(xs)
                    nc.vector.tensor_scalar(
                        out=t_u16[:, lo:hi], in0=xs_u16[j % 2][:, lo:hi],
                        scalar1=0x7C00, scalar2=None, op0=alu.bitwise_and,
                    )
                    # t = (max(t_e, bits(1.0)) + 0x2400: PO2 of max(|xs|,1)
                    # times 512, as an exponent-field add
                    nc.vector.tensor_scalar(
                        out=t_u16[:, lo:hi], in0=t_u16[:, lo:hi],
                        scalar1=0x3C00, scalar2=0x2400,
                        op0=alu.max, op1=alu.add,
                    )
                    # s = sign pairs
                    nc.vector.tensor_scalar(
                        out=s_i32[:, l2:h2], in0=xs_i32[j % 2][:, l2:h2],
                        scalar1=-2147450880, scalar2=None, op0=alu.bitwise_and,
                    )
                    # t' = t | s  (copysign, in place over t)
                    nc.vector.tensor_tensor(
                        out=t_i32[:, l2:h2], in0=t_i32[:, l2:h2],
                        in1=s_i32[:, l2:h2], op=alu.bitwise_or,
                    )
                    # c = RN16(xs + t'): rounds xs at the grid quantum
                    if half is None:
                        # c/y in feature halves: y[0:2048] lands ~2.2us
                        # earlier so ACT can start dq (and the h0 transpose)
                        # sooner; sFR +1 per half keeps the counting intact
                        for hh in range(2):
                            ha, hb = hh * (K // 2), (hh + 1) * (K // 2)
                            nc.vector.tensor_tensor(
                                out=y_sb[j % 2][:, ha:hb],
                                in0=xs_sb[j % 2][:, ha:hb],
                                in1=t_sb[:, ha:hb], op=alu.add,
                            )
                            nc.vector.tensor_tensor(
                                out=y_sb[j % 2][:, ha:hb],
                                in0=y_sb[j % 2][:, ha:hb],
                                in1=t_sb[:, ha:hb], op=alu.subtract,
                            ).then_inc(sFR, 1)
                    else:
                        nc.vector.tensor_tensor(
                            out=y_sb[j % 2][:, lo:hi], in0=xs_sb[j % 2][:, lo:hi],
                            in1=t_sb[:, lo:hi], op=alu.add,
                        )
                        # y = c - t'  (exact; signed snap)
                        nc.vector.tensor_tensor(
                            out=y_sb[j % 2][:, lo:hi], in0=y_sb[j % 2][:, lo:hi],
                            in1=t_sb[:, lo:hi], op=alu.subtract,
                        ).then_inc(sFR, 1)

                stats(0, half=0)
                stats(0, half=1)
                front(0, half=0)
                stats(1)
                front(0, half=1)
                stats(2)
                for i in range(1, NTL):
                    front(i)
                    if i + 2 == 3:
                        stats(3, half=0)
                        stats(3, half=1)
                    else:
                        stats(i + 2)

            @block.gpsimd
            def _(eng):
                def xs_op(i, gl, gh):
                    r_b = rr_sb[i % 2][:, gl:gh].unsqueeze(2).broadcast_to(
                        (TT, gh - gl, GS)
                    )
                    nc.gpsimd.tensor_tensor(
                        out=xs_sb[i % 2].rearrange(
                            "p (g s) -> p g s", s=GS)[:, gl:gh, :],
                        in0=x_sb[i % 3][:, gl:gh, :],
                        in1=r_b, op=alu.mult,
                    )

                # tile0 in halves (fill latency)
                eng.wait_ge(sX0A, 16)
                eng.wait_ge(sRR, 1)
                xs_op(0, 0, G // 2)
                nc.gpsimd.drain().then_inc(sXS, 1)
                eng.wait_ge(sX0B, 16)
                eng.wait_ge(sRR, 2)
                xs_op(0, G // 2, G)
                nc.gpsimd.drain().then_inc(sXS, 1)
                for i in range(1, NTL):
                    wait_x(eng, i)
                    eng.wait_ge(sRR, 2 * (i + 1))
                    if i >= 2:
                        eng.wait_ge(sFR, 2 * (i - 1))  # xs buf free
                    xs_op(i, 0, G)
                    nc.gpsimd.drain().then_inc(sXS, 2)

            @block.scalar
            def _(eng):
                def mk_dq(j):
                    if j >= NTL:
                        return
                    # every front now incs sFR +1 at its half mark -- dq
                    # groups 0-15 only need the first half of y
                    eng.wait_ge(sFR, 2 * j + 1)
                    if j >= 2:
                        wait_tp(eng, j - 2, full=True)    # dq buf free
                    for g in range(G):
                        if g == 16:
                            eng.wait_ge(sFR, 2 * (j + 1))
                        nc.scalar.activation(
                            out=dq3[j % 2][:, g, :], in_=y3[j % 2][:, g, :],
                            func=AF.Copy, scale=sc_sb[j % 4][:, g:g + 1],
                        )
                        if g % 16 == 15:
                            nc.scalar.drain().then_inc(sDU, 16)
                            if j > 0:
                                stp = sTPa if g == 15 else sTPb
                                if g == 15 and j >= 2:
                                    eng.wait_ge(sMM, j - 1)   # dqt buf free
                                eng.wait_ge(stp, 16 * j)      # sem-order guard
                                eng.dma_start(
                                    out=dqt_sb[j % 2][:, g - 15:g + 1, :],
                                    in_=dq_sb[j % 2][:, 128 * (g - 15):128 * (g + 1)],
                                    transpose=True,
                                ).then_inc(stp, 16)

                def out_copy(j):
                    if j < 0 or j >= NTL:
                        return
                    eng.wait_ge(sMM, j + 1)
                    sod = sODa if j % 2 == 0 else sODb
                    if j >= 2:
                        eng.wait_ge(sod, 16 * (j // 2))  # o_sb free
                    nc.scalar.activation(
                        out=o_sb[j % 2][:, :], in_=pout_ps[j % 2][:, :],
                        func=AF.Copy,
                    )
                    nc.scalar.drain().then_inc(sOC, 1)
                    eng.dma_start(
                        out=out_d[j * TT:(j + 1) * TT, :], in_=o_sb[j % 2][:, :]
                    ).then_inc(sod, 16)

                for i in range(NTL):
                    mk_dq(i)
                    out_copy(i - 2)
                out_copy(NTL - 2)
                out_copy(NTL - 1)

            @block.tensor
            def _(eng):
                # self-paced warmup: keeps PE continuously busy (and the
                # p-state ramped) until tile0's transposes land.  Reads
                # whatever is in SBUF; results go to a scratch PSUM bank.
                eng.wait_ge(sV, 1)
                for p in range(WARM):
                    if p >= 2:
                        eng.wait_ge(sWU, 2 * p - 4)
                    for _ in range(2):
                        nc.tensor.matmul(
                            warm_ps[:, :],
                            lhsT=warm_sb[:, 0:128],
                            rhs=warm_sb[:, :],
                            start=True, stop=True,
                        ).then_inc(sWU, 1)

                for i in range(NTL):
                    if i >= 2:
                        eng.wait_ge(sOC, i - 1)    # pout buf free
                    if i == 0:
                        # coarse start: everything tile0 needs is resident
                        # (the warmup stream keeps the PE busy until here)
                        eng.wait_ge(sWT, 128)
                        eng.wait_ge(sC, 16)
                        eng.wait_ge(sV, 1)
                    for b in range(G):
                        if b == 0:
                            wait_tp(eng, i, full=False)
                        elif b == 16:
                            wait_tp(eng, i, full=True)
                        for hf in range(2):
                            nc.tensor.matmul(
                                pout_ps[i % 2][:, hf * 512:(hf + 1) * 512],
                                lhsT=dqt_sb[i % 2][:, b, :],
                                rhs=wt_sb[:, b, hf * 512:(hf + 1) * 512],
                                start=(b == 0),
                                stop=False,
                            )
                    for hf in range(2):
                        ins = nc.tensor.matmul(
                            pout_ps[i % 2][:, hf * 512:(hf + 1) * 512],
                            lhsT=ones_sb[:, :],
                            rhs=bias_sb[:, hf * 512:(hf + 1) * 512],
                            start=False,
                            stop=True,
                        )
                        if hf == 1:
                            ins.then_inc(sMM, 1)

    return nc


def _get_nc():
    if "nc" not in _CACHE:
        _CACHE["nc"] = _build_bass()
    return _CACHE["nc"]


def make_in_maps(x, weight, bias):
    x2 = np.ascontiguousarray(np.asarray(x, dtype=np.float32).reshape(TOK, K))
    wt = np.ascontiguousarray(np.asarray(weight, dtype=np.float32).T).astype(
        np.float16
    )
    bias_h = np.asarray(bias, dtype=np.float32).reshape(1, M).astype(np.float16)
    return [
        {"x": x2[i * TPC:(i + 1) * TPC], "wt": wt, "bias": bias_h}
        for i in range(N_CORES)
    ]


def kernel(x, weight, bias, grid=None, **_ignored):
    from concourse.bass_utils import run_bass_kernel_spmd

    nc = _get_nc()
    in_maps = make_in_maps(x, weight, bias)
    res = run_bass_kernel_spmd(nc, in_maps, core_ids=list(range(N_CORES)))
    out = np.concatenate([res.results[i]["out"] for i in range(N_CORES)], axis=0)
    return out.reshape(4, 4096, M).astype(np.float32)


# revision 45
# speedup vs baseline: 1.0261x; 1.0001x over previous
"""ANT_Linear fused kernel for 8 TRN2 NeuronCores (raw Bass, manual sems).

out = fakequant(x) @ W.T + bias; per-128-group absmax scaling of x snapped to
the 15-level e2m1 ('flint') grid.  Data-parallel over tokens: 2048/core,
16 tiles of [128 tokens, 4096 features].

v3: PE-saturating schedule with a signed magic-add snap:

  xs = RN16(x*(6/absmax))               (Pool broadcast mult)
  t' = copysign(512*2^floor(log2(max(|xs|,1))), xs)
       via uint16-lane bit ops: exp-mask, clamp, +9<<10, or-sign  (DVE)
  c  = RN16(xs + t')                    (DVE: rounds xs at the grid quantum)
  y  = c - t'                           (DVE, exact: y = snap(xs) signed)
  dq = y * scale                        (ACT per-group Copy-with-scale)
  dqT = dma_transpose(dq); out = dqT.T @ W.T + bias on PE (fp16 matmuls)

Engine busy per [128,4096] tile (cost-model ns):
  DVE : reduce 4297 + sc/rr 171 + t_e/t/s 3x1127 + t' 2163 + c/y 2x2194
        -> ~14.7us/tile  <- pipeline bound
  PE  : 64 fp16 matmuls + 2 bias matmuls = 14.1us
  ACT : dq 32x292 + psum->sbuf out copy = 10.4us;  Pool: xs 8.3us
  DMA : x 5.8 + 2 transpose halves 3.6 + out 1.5 = 10.9us (single queue)

Schedule notes:
 - A self-paced warmup stream of dummy matmuls (pairs, self-incrementing
   sem, 2-pair lookahead) keeps the PE engine continuously busy -- and the
   cost model's p-state ramp warm -- until tile0's inputs are resident
   (~51us: the FIFO DMA queue must carry x0..x3 + all weights + bias +
   tile0's transposes first).  Tile0 then starts coarse (all waits up
   front) and the stream stays gap-free; the quant chain runs ~1.5 tiles
   ahead of the PE throughout.
 - tile0's x-load/stats/xs/front/dq are split in halves to shorten the
   fill; the DMA issue order is hand-choreographed (x2 gated on rr0, the
   last 4 weight chunks gated on tile0's transposes, x4 gated on dq(1))
   because the cost model serializes all DMAs on one engine FIFO.
 - DMA completions are NOT ordered across engines: every wait must
   identify one specific DMA.  Hence per-buffer-slot x sems, per-half
   sems for the split x0/x3, h0/h1 transpose sems (sTPa/sTPb), out-DMA
   parity sems (sODa/sODb), and predecessor waits before reusing a sem.
 - Iterative DVE ops (tensor_reduce, reciprocal) need an explicit
   drain() before a dependent consumer (HW RAW hazard -- verified: NaNs
   without).  Simple ALU chains (ts/tt) are safe drain-free, and sem
   incs ride on the last compute op instead of a trailing drain.

Numerics: bit-exact with the v2 baseline except the PO2 magic constant
fixes v2's snap at |xs| in {1.997..1.999, 3.994..3.998} (v2 rounded these
to 1.5/3.0 instead of 2.0/4.0) -- rel err improves 1.43e-2 -> 1.11e-2.
"""

import numpy as np

N_CORES = 8
TOK = 4 * 4096
TPC = TOK // N_CORES    # 2048
K = 4096
M = 1024
GS = 128
G = K // GS             # 32
TT = 128
NT = TPC // TT          # 16

WARM = 112              # warmup matmul pairs

_CACHE = {}


def _register_snap_ops():
    """Register two fused custom-DVE ops implementing the signed magic-add
    snap in one pass each (f32 bit tricks inside the DVE pipe):
      ANT_SNAP_C: c = RN16(xs + t'(xs))
      ANT_SNAP_Y: y = c - t'(xs)        (sign taken from c; sign(c)==sign(xs))
    with t'(u) = copysign(512 * 2^floor(log2(max(|u|,1))), u).
    """
    if "snap_ops" in _CACHE:
        return _CACHE["snap_ops"]
    import concourse.dve_ops as dops
    from concourse.dve_spec import (AluOp, Bin, C0, C1, C2, Spec, Src0, Src1,
                                    Zero, _has_src1, lower, maxx, select)
    from concourse.dve_uop import DveOpSpec

    def _tmag_np(src, s1, imm2):
        b = (src.astype(np.float32).view(np.int32) & 0x7F800000).view(
            np.float32)
        return np.maximum(b, np.float32(s1)) * np.float32(imm2)

    def ref_snap_c(in0, in1, s0, s1, imm2):
        x = in0.astype(np.float32)
        t = _tmag_np(x, s1, imm2)
        return x + np.where(x < 0, -t, t).astype(np.float32)

    def ref_snap_y(in0, in1, s0, s1, imm2):
        c = in0.astype(np.float32)
        t = _tmag_np(in1, s1, imm2)
        return c - np.where(c < 0, -t, t).astype(np.float32)

    _tm0 = maxx(Bin(AluOp.BITWISE_AND, Src0, C0), C1) * C2
    spec_c = Spec(body=Src0 + select(Src0 < Zero, Zero - _tm0, _tm0),
                  reference=ref_snap_c)
    _tm1 = maxx(Bin(AluOp.BITWISE_AND, Src1, C0), C1) * C2
    spec_y = Spec(body=Src0 - select(Src0 < Zero, Zero - _tm1, _tm1),
                  reference=ref_snap_y)

    ops = []
    for name, spec in (("ANT_SNAP_C", spec_c), ("ANT_SNAP_Y", spec_y)):
        if name in dops._SUB_OPCODE_FOR_NAME:
            ops.append(next(o for o in dops.OPS if o.name == name))
            continue
        row = max(dops._SUB_OPCODE_FOR_NAME.values()) + 1
        assert row < 0x20
        dops._SUB_OPCODE_FOR_NAME[name] = row
        shas = {}
        for ver in ("v3", "v4"):
            uops = lower(spec, ver=ver)
            sp = DveOpSpec(name=name, opcode=row, uops=uops,
                           rd1_en=_has_src1(spec))
            shas[ver] = sp.sha(ver)
        op = dops.DveOp(name, spec, subdim=False, uops_sha=shas)
        dops.OPS.append(op)
        dops.CUSTOM_DVE_SPECS[name] = spec
        ops.append(op)
    _CACHE["snap_ops"] = tuple(ops)
    return _CACHE["snap_ops"]


def _build_bass(nt=NT):
    from contextlib import ExitStack

    import concourse.bass as bass
    import concourse.mybir as mybir

    dt = mybir.dt
    alu = mybir.AluOpType
    AF = mybir.ActivationFunctionType

    SNAP_C, SNAP_Y = _register_snap_ops()

    NTL = nt
    nc = bass.Bass()
    x_d = nc.declare_dram_parameter("x", [NTL * TT, K], dt.float32, isOutput=False)
    wt_d = nc.declare_dram_parameter("wt", [K, M], dt.float16, isOutput=False)
    b_d = nc.declare_dram_parameter("bias", [1, M], dt.float16, isOutput=False)
    out_d = nc.declare_dram_parameter("out", [NTL * TT, M], dt.float32, isOutput=True)

    x_t4 = x_d.rearrange("(n p) (g s) -> n p g s", p=TT, s=GS)   # [16,128,32,128]
    wt_t3 = wt_d.rearrange("(b p) m -> p b m", p=128)            # [128,32,1024]

    ctx = ExitStack()
    with ctx:
        sb = lambda name, shape, d: ctx.enter_context(nc.sbuf_tensor(name, shape, d))
        ps = lambda name, shape, d: ctx.enter_context(nc.psum_tensor(name, shape, d))
        sem = lambda name: ctx.enter_context(nc.semaphore(name))

        wt_sb = sb("wt_sb", [128, G, M], dt.float16)            # 8 MiB resident
        bias_sb = sb("bias_sb", [1, M], dt.float16)
        ones_sb = sb("ones_sb", [1, TT], dt.float16)

        x_sb = [sb(f"x_sb{k}", [TT, G, GS], dt.float32) for k in range(3)]
        xs_sb = [sb(f"xs_sb{k}", [TT, K], dt.float16) for k in range(2)]
        t_sb = sb("t_sb", [TT, K], dt.float16)
        s_sb = sb("s_sb", [TT, K], dt.float16)
        y_sb = [sb(f"y_sb{k}", [TT, K], dt.float16) for k in range(2)]
        dq_sb = [sb(f"dq_sb{k}", [TT, K], dt.float16) for k in range(2)]
        dqt_sb = [sb(f"dqt_sb{k}", [128, G, TT], dt.float16) for k in range(2)]
        o_sb = [sb(f"o_sb{k}", [TT, M], dt.float32) for k in range(2)]
        warm_sb = sb("warm_sb", [TT, 512], dt.float16)
        amax_sb = [sb(f"amax_sb{k}", [TT, G], dt.float32) for k in range(2)]
        sc_sb = [sb(f"sc_sb{k}", [TT, G], dt.float32) for k in range(4)]
        rr_sb = [sb(f"rr_sb{k}", [TT, G], dt.float32) for k in range(2)]

        pout_ps = [ps(f"pout_ps{k}", [TT, M], dt.float32) for k in range(2)]
        warm_ps = ps("warm_ps", [TT, 512], dt.float32)

        sC = sem("sC")     # bias DMA done (+16)
        sV = sem("sV")     # ones/warm memset done
        # x-load sems: one per buffer slot (+ dedicated sems for the four
        # half-loads of x0/x3) so a sem value always identifies ONE DMA --
        # DMA completions are not ordered across engines.
        sXA = sem("sXA")   # slot0 whole loads: x6, x9, x12, x15
        sXB = sem("sXB")   # slot1 loads: x1, x4, x7, x10, x13
        sXC = sem("sXC")   # slot2 loads: x2, x5, x8, x11, x14
        sX0A = sem("sX0A")  # x0 first half
        sX0B = sem("sX0B")  # x0 second half
        sX3A = sem("sX3A")  # x3 first half
        sX3B = sem("sX3B")  # x3 second half
        sWT = sem("sWT")   # weight chunk in (+16/chunk)
        sRR = sem("sRR")   # stats done (amax/sc/rr) (+1/half for tile0, +2/tile)
        sXS = sem("sXS")   # Pool xs done (+1/half tile0, +2/tile)
        sFR = sem("sFR")   # DVE front chain done (+1/half tile0, +2/tile)
        sDU = sem("sDU")   # ACT dq half done (+16/half)
        sTPa = sem("sTPa")  # transpose h0 done (+16/tile)
        sTPb = sem("sTPb")  # transpose h1 done (+16/tile)
        sMM = sem("sMM")   # PE tile done (+1/tile)
        sOC = sem("sOC")   # ACT out copy done (+1/tile)
        sODa = sem("sODa")  # out DMA done, even tiles (+16)
        sODb = sem("sODb")  # out DMA done, odd tiles (+16)
        sWU = sem("sWU")   # warmup pacing (+1/warmup mm)
        sMH = sem("sMH")   # last tile: bias half-0 done

        def x_sem(i):
            """(sem, value) identifying completion of whole-load x(i)."""
            if i % 3 == 0:
                return sXA, 16 * (i // 3 - 1)
            if i % 3 == 1:
                return sXB, 16 * ((i - 1) // 3 + 1)
            return sXC, 16 * ((i - 2) // 3 + 1)

        def wait_x(eng, j):
            """Wait for x(j) fully loaded."""
            if j == 0:
                eng.wait_ge(sX0A, 16)
                eng.wait_ge(sX0B, 16)
            elif j == 3:
                eng.wait_ge(sX3A, 16)
                eng.wait_ge(sX3B, 16)
            else:
                s, v = x_sem(j)
                eng.wait_ge(s, v)

        xs_i32 = [xs_sb[k].bitcast(dt.int32) for k in range(2)]
        xs_u16 = [xs_sb[k].bitcast(dt.uint16) for k in range(2)]
        t_i32 = t_sb.bitcast(dt.int32)
        t_u16 = t_sb.bitcast(dt.uint16)
        s_i32 = s_sb.bitcast(dt.int32)
        y3 = [y_sb[k].rearrange("p (g s) -> p g s", s=GS) for k in range(2)]
        dq3 = [dq_sb[k].rearrange("p (g s) -> p g s", s=GS) for k in range(2)]

        def wait_tp(eng, j, full):
            """Wait for tile j's transpose h0 (full=False) or both halves."""
            eng.wait_ge(sTPa, 16 * (j + 1))
            if full:
                eng.wait_ge(sTPb, 16 * (j + 1))

        with nc.Block() as block:

            @block.sync
            def _(eng):
                # FIFO DMA queue choreography (see docstring).
                eng.dma_start(
                    out=x_sb[0][:, 0:16, :], in_=x_t4[0][:, 0:16, :]
                ).then_inc(sX0A, 16)
                eng.dma_start(
                    out=x_sb[0][:, 16:32, :], in_=x_t4[0][:, 16:32, :]
                ).then_inc(sX0B, 16)
                eng.dma_start(out=x_sb[1][:, :, :], in_=x_t4[1]).then_inc(sXB, 16)
                eng.dma_start(out=bias_sb[:, :], in_=b_d[:, :]).then_inc(sC, 16)
                for c in range(2):
                    eng.dma_start(
                        out=wt_sb[:, 4 * c:4 * (c + 1), :],
                        in_=wt_t3[:, 4 * c:4 * (c + 1), :],
                    ).then_inc(sWT, 16)
                eng.wait_ge(sRR, 2)                # x2 right when stats need it
                eng.dma_start(out=x_sb[2][:, :, :], in_=x_t4[2]).then_inc(sXC, 16)
                for c in range(2, 6):
                    eng.dma_start(
                        out=wt_sb[:, 4 * c:4 * (c + 1), :],
                        in_=wt_t3[:, 4 * c:4 * (c + 1), :],
                    ).then_inc(sWT, 16)
                eng.wait_ge(sDU, 16)               # tile0 dq half ready
                eng.dma_start(
                    out=dqt_sb[0][:, 0:16, :], in_=dq_sb[0][:, 0:2048],
                    transpose=True,
                ).then_inc(sTPa, 16)
                eng.wait_ge(sDU, 32)
                eng.dma_start(
                    out=dqt_sb[0][:, 16:32, :], in_=dq_sb[0][:, 2048:4096],
                    transpose=True,
                ).then_inc(sTPb, 16)
                eng.dma_start(
                    out=x_sb[0][:, 0:16, :], in_=x_t4[3][:, 0:16, :]
                ).then_inc(sX3A, 16)
                for c in range(6, 8):
                    eng.dma_start(
                        out=wt_sb[:, 4 * c:4 * (c + 1), :],
                        in_=wt_t3[:, 4 * c:4 * (c + 1), :],
                    ).then_inc(sWT, 16)
                eng.dma_start(
                    out=x_sb[0][:, 16:32, :], in_=x_t4[3][:, 16:32, :]
                ).then_inc(sX3B, 16)
                for i in range(4, NTL):
                    if i == 4:
                        # don't head-of-line block tile1's transposes
                        eng.wait_ge(sDU, 64)
                    eng.wait_ge(sXS, 2 * (i - 2))  # x buf free (x3 ring)
                    s, v = x_sem(i)
                    if v > 16:
                        eng.wait_ge(s, v - 16)     # slot predecessor done
                    eng.dma_start(
                        out=x_sb[i % 3][:, :, :], in_=x_t4[i]
                    ).then_inc(s, 16)

            @block.vector
            def _(eng):
                nc.vector.memset(warm_sb[:, :], 0.5)
                nc.vector.drain().then_inc(sV, 1)
                nc.vector.memset(ones_sb[:, :], 1.0)
                nc.vector.drain().then_inc(sV, 1)

                def stats(j, half=None):
                    if j >= NTL:
                        return
                    gl, gh = (0, G) if half is None else (
                        (0, G // 2) if half == 0 else (G // 2, G))
                    if j == 0:
                        eng.wait_ge(sX0A if half == 0 else sX0B, 16)
                    elif j == 3:
                        eng.wait_ge(sX3A if half == 0 else sX3B, 16)
                    else:
                        wait_x(eng, j)
                    if j >= 4:
                        # sc buf (x4): dq(j-4) groups all scaled
                        eng.wait_ge(sDU, 32 * (j - 3))
                    nc.vector.tensor_reduce(
                        out=amax_sb[j % 2][:, gl:gh],
                        in_=x_sb[j % 3][:, gl:gh, :],
                        axis=mybir.AxisListType.X, op=alu.max,
                        apply_absolute_value=True,
                    )
                    nc.vector.drain()
                    # scale = max(amax/6, tiny); rr = 1/scale
                    nc.vector.tensor_scalar(
                        out=sc_sb[j % 4][:, gl:gh], in0=amax_sb[j % 2][:, gl:gh],
                        scalar1=1.0 / 6.0, scalar2=1e-30,
                        op0=alu.mult, op1=alu.max,
                    )
                    nc.vector.drain()
                    nc.vector.reciprocal(
                        out=rr_sb[j % 2][:, gl:gh], in_=sc_sb[j % 4][:, gl:gh]
                    ).then_inc(sRR, 2 if half is None else 1)

                def front(j, half=None):
                    if j < 0 or j >= NTL:
                        return
                    lo, hi = (0, K) if half is None else (
                        (0, K // 2) if half == 0 else (K // 2, K))
                    eng.wait_ge(sXS, 2 * j + 1 + (half != 0))
                    if j >= 2:
                        eng.wait_ge(sDU, 32 * (j - 1))  # y buf free
                    l2, h2 = lo // 2, hi // 2
                    # t_e = exp_bits(xs)
                    nc.vector.tensor_scalar(
                        out=t_u16[:, lo:hi], in0=xs_u16[j % 2][:, lo:hi],
                        scalar1=0x7C00, scalar2=None, op0=alu.bitwise_and,
                    )
                    # t = (max(t_e, bits(1.0)) + 0x2400: PO2 of max(|xs|,1)
                    # times 512, as an exponent-field add
                    nc.vector.tensor_scalar(
                        out=t_u16[:, lo:hi], in0=t_u16[:, lo:hi],
                        scalar1=0x3C00, scalar2=0x2400,
                        op0=alu.max, op1=alu.add,
                    )
                    # s = sign pairs
                    nc.vector.tensor_scalar(
                        out=s_i32[:, l2:h2], in0=xs_i32[j % 2][:, l2:h2],
                        scalar1=-2147450880, scalar2=None, op0=alu.bitwise_and,
                    )
                    # t' = t | s  (copysign, in place over t)
                    nc.vector.tensor_tensor(
                        out=t_i32[:, l2:h2], in0=t_i32[:, l2:h2],
                        in1=s_i32[:, l2:h2], op=alu.bitwise_or,
                    )
                    # c = RN16(xs + t'): rounds xs at the grid quantum
                    if half is None:
                        # c/y in feature halves: y[0:2048] lands ~2.2us
                        # earlier so ACT can start dq (and the h0 transpose)
                        # sooner; sFR +1 per half keeps the counting intact
                        for hh in range(2):
                            ha, hb = hh * (K // 2), (hh + 1) * (K // 2)
                            nc.vector.tensor_tensor(
                                out=y_sb[j % 2][:, ha:hb],
                                in0=xs_sb[j % 2][:, ha:hb],
                                in1=t_sb[:, ha:hb], op=alu.add,
                            )
                            nc.vector.tensor_tensor(
                                out=y_sb[j % 2][:, ha:hb],
                                in0=y_sb[j % 2][:, ha:hb],
                                in1=t_sb[:, ha:hb], op=alu.subtract,
                            ).then_inc(sFR, 1)
                    else:
                        nc.vector.tensor_tensor(
                            out=y_sb[j % 2][:, lo:hi], in0=xs_sb[j % 2][:, lo:hi],
                            in1=t_sb[:, lo:hi], op=alu.add,
                        )
                        # y = c - t'  (exact; signed snap)
                        nc.vector.tensor_tensor(
                            out=y_sb[j % 2][:, lo:hi], in0=y_sb[j % 2][:, lo:hi],
                            in1=t_sb[:, lo:hi], op=alu.subtract,
                        ).then_inc(sFR, 1)

                stats(0, half=0)
                stats(0, half=1)
                front(0, half=0)
                stats(1)
                front(0, half=1)
                stats(2)
                for i in range(1, NTL):
                    front(i)
                    if i + 2 == 3:
                        stats(3, half=0)
                        stats(3, half=1)
                    else:
                        stats(i + 2)

            @block.gpsimd
            def _(eng):
                def xs_op(i, gl, gh):
                    r_b = rr_sb[i % 2][:, gl:gh].unsqueeze(2).broadcast_to(
                        (TT, gh - gl, GS)
                    )
                    nc.gpsimd.tensor_tensor(
                        out=xs_sb[i % 2].rearrange(
                            "p (g s) -> p g s", s=GS)[:, gl:gh, :],
                        in0=x_sb[i % 3][:, gl:gh, :],
                        in1=r_b, op=alu.mult,
                    )

                # tile0 in halves (fill latency)
                eng.wait_ge(sX0A, 16)
                eng.wait_ge(sRR, 1)
                xs_op(0, 0, G // 2)
                nc.gpsimd.drain().then_inc(sXS, 1)
                eng.wait_ge(sX0B, 16)
                eng.wait_ge(sRR, 2)
                xs_op(0, G // 2, G)
                nc.gpsimd.drain().then_inc(sXS, 1)
                for i in range(1, NTL):
                    wait_x(eng, i)
                    eng.wait_ge(sRR, 2 * (i + 1))
                    if i >= 2:
                        eng.wait_ge(sFR, 2 * (i - 1))  # xs buf free
                    xs_op(i, 0, G)
                    nc.gpsimd.drain().then_inc(sXS, 2)

            @block.scalar
            def _(eng):
                def mk_dq(j):
                    if j >= NTL:
                        return
                    # every front now incs sFR +1 at its half mark -- dq
                    # groups 0-15 only need the first half of y
                    eng.wait_ge(sFR, 2 * j + 1)
                    if j >= 2:
                        wait_tp(eng, j - 2, full=True)    # dq buf free
                    for g in range(G):
                        if g == 16:
                            eng.wait_ge(sFR, 2 * (j + 1))
                        nc.scalar.activation(
                            out=dq3[j % 2][:, g, :], in_=y3[j % 2][:, g, :],
                            func=AF.Copy, scale=sc_sb[j % 4][:, g:g + 1],
                        )
                        if g % 16 == 15:
                            nc.scalar.drain().then_inc(sDU, 16)
                            if j > 0:
                                stp = sTPa if g == 15 else sTPb
                                if g == 15 and j >= 2:
                                    eng.wait_ge(sMM, j - 1)   # dqt buf free
                                eng.wait_ge(stp, 16 * j)      # sem-order guard
                                eng.dma_start(
                                    out=dqt_sb[j % 2][:, g - 15:g + 1, :],
                                    in_=dq_sb[j % 2][:, 128 * (g - 15):128 * (g + 1)],
                                    transpose=True,
                                ).then_inc(stp, 16)

                def out_copy(j):
                    if j < 0 or j >= NTL:
                        return
                    sod = sODa if j % 2 == 0 else sODb
                    if j == NTL - 1:
                        # last tile: copy/store column halves as each bias
                        # matmul retires -- shortens the serial tail
                        eng.wait_ge(sMH, 1)
                        nc.scalar.activation(
                            out=o_sb[j % 2][:, 0:512],
                            in_=pout_ps[j % 2][:, 0:512], func=AF.Copy,
                        )
                        nc.scalar.drain()
                        eng.dma_start(
                            out=out_d[j * TT:(j + 1) * TT, 0:512],
                            in_=o_sb[j % 2][:, 0:512],
                        ).then_inc(sod, 16)
                        eng.wait_ge(sMM, j + 1)
                        nc.scalar.activation(
                            out=o_sb[j % 2][:, 512:1024],
                            in_=pout_ps[j % 2][:, 512:1024], func=AF.Copy,
                        )
                        nc.scalar.drain().then_inc(sOC, 1)
                        eng.dma_start(
                            out=out_d[j * TT:(j + 1) * TT, 512:1024],
                            in_=o_sb[j % 2][:, 512:1024],
                        ).then_inc(sod, 16)
                        return
                    eng.wait_ge(sMM, j + 1)
                    if j >= 2:
                        eng.wait_ge(sod, 16 * (j // 2))  # o_sb free
                    nc.scalar.activation(
                        out=o_sb[j % 2][:, :], in_=pout_ps[j % 2][:, :],
                        func=AF.Copy,
                    )
                    nc.scalar.drain().then_inc(sOC, 1)
                    eng.dma_start(
                        out=out_d[j * TT:(j + 1) * TT, :], in_=o_sb[j % 2][:, :]
                    ).then_inc(sod, 16)

                for i in range(NTL):
                    mk_dq(i)
                    out_copy(i - 2)
                out_copy(NTL - 2)
                out_copy(NTL - 1)

            @block.tensor
            def _(eng):
                # self-paced warmup: keeps PE continuously busy (and the
                # p-state ramped) until tile0's transposes land.  Reads
                # whatever is in SBUF; results go to a scratch PSUM bank.
                eng.wait_ge(sV, 1)
                for p in range(WARM):
                    if p >= 2:
                        eng.wait_ge(sWU, 2 * p - 4)
                    for _ in range(2):
                        nc.tensor.matmul(
                            warm_ps[:, :],
                            lhsT=warm_sb[:, 0:128],
                            rhs=warm_sb[:, :],
                            start=True, stop=True,
                        ).then_inc(sWU, 1)

                for i in range(NTL):
                    if i >= 2:
                        eng.wait_ge(sOC, i - 1)    # pout buf free
                    if i == 0:
                        # coarse start: everything tile0 needs is resident
                        # (the warmup stream keeps the PE busy until here)
                        eng.wait_ge(sWT, 128)
                        eng.wait_ge(sC, 16)
                        eng.wait_ge(sV, 2)
                    for b in range(G):
                        if b == 0:
                            wait_tp(eng, i, full=False)
                        elif b == 16:
                            wait_tp(eng, i, full=True)
                        for hf in range(2):
                            nc.tensor.matmul(
                                pout_ps[i % 2][:, hf * 512:(hf + 1) * 512],
                                lhsT=dqt_sb[i % 2][:, b, :],
                                rhs=wt_sb[:, b, hf * 512:(hf + 1) * 512],
                                start=(b == 0),
                                stop=False,
                            )
                    for hf in range(2):
                        ins = nc.tensor.matmul(
                            pout_ps[i % 2][:, hf * 512:(hf + 1) * 512],
                            lhsT=ones_sb[:, :],
                            rhs=bias_sb[:, hf * 512:(hf + 1) * 512],
                            start=False,
                            stop=True,
                        )
                        if hf == 0 and i == NTL - 1:
                            ins.then_inc(sMH, 1)
                        elif hf == 1:
                            ins.then_inc(sMM, 1)

    return nc


def _get_nc():
    if "nc" not in _CACHE:
        _CACHE["nc"] = _build_bass()
    return _CACHE["nc"]


def make_in_maps(x, weight, bias):
    x2 = np.ascontiguousarray(np.asarray(x, dtype=np.float32).reshape(TOK, K))
    wt = np.ascontiguousarray(np.asarray(weight, dtype=np.float32).T).astype(
        np.float16
    )
    bias_h = np.asarray(bias, dtype=np.float32).reshape(1, M).astype(np.float16)
    return [
        {"x": x2[i * TPC:(i + 1) * TPC], "wt": wt, "bias": bias_h}
        for i in range(N_CORES)
    ]


def kernel(x, weight, bias, grid=None, **_ignored):
    from concourse.bass_utils import run_bass_kernel_spmd

    nc = _get_nc()
    in_maps = make_in_maps(x, weight, bias)
    res = run_bass_kernel_spmd(nc, in_maps, core_ids=list(range(N_CORES)))
    out = np.concatenate([res.results[i]["out"] for i in range(N_CORES)], axis=0)
    return out.reshape(4, 4096, M).astype(np.float32)
